# revision 18
# baseline (speedup 1.0000x reference)
"""MiniMaxText01 linear attention (lightning attention) prefill layer on 8 TRN2 NeuronCores.

Strategy: tensor-parallel over heads (4 heads/core) for qkv+gate+attention;
AllToAll to switch to sequence-parallel for the output projection;
ReduceScatter for the RMSNorm variance. See spec sharding_hint.
"""
import os
import sys
import math

sys.path.insert(0, "/opt/trn_rl_repo")

import numpy as np
import ml_dtypes

import concourse.bass as bass
import concourse.mybir as mybir
import concourse.tile as tile
from concourse import bacc
from concourse.bass_utils import run_bass_kernel_spmd

# problem constants (hardcoded per contract)
H = 4096
INNER = 4096
NH = 32
HD = 128
SEQ = 4096
BLOCK = 256
EPS = 1e-5
P = 128
W = 8                    # cores
HPC = NH // W            # heads per core = 4
MPC = 3 * HD * HPC       # qkv rows per core = 1536
JPC = HD * HPC           # inner cols per core = 512
SSH = SEQ // W           # seq shard = 512
KO = H // P              # 32 k-subtiles
NB = SEQ // BLOCK        # 16 blocks
SB = SEQ // P            # 32 sub-blocks of 128

F32 = mybir.dt.float32
F32R = mybir.dt.float32r
BF16 = mybir.dt.bfloat16
AF = mybir.ActivationFunctionType
ALU = mybir.AluOpType


def _finish(nc):
    return nc


def _build_program():
    nc = bacc.Bacc("TRN2", target_bir_lowering=False, debug=False, num_devices=W)

    # ---- I/O ----
    xT = nc.dram_tensor("xT", [H, SEQ], BF16, kind="ExternalInput")
    wqkvT = nc.dram_tensor("wqkvT", [H, MPC], BF16, kind="ExternalInput")
    wgateT = nc.dram_tensor("wgateT", [H, JPC], BF16, kind="ExternalInput")
    woutT = nc.dram_tensor("woutT", [INNER, H], F32R, kind="ExternalInput")
    qdec = nc.dram_tensor("qdec", [P, HPC, BLOCK], F32, kind="ExternalInput")
    kdec = nc.dram_tensor("kdec", [P, HPC, 2], F32, kind="ExternalInput")
    maskT = nc.dram_tensor("maskT", [P, HPC, 2, BLOCK], F32, kind="ExternalInput")
    blkdec = nc.dram_tensor("blkdec", [P, HPC], F32, kind="ExternalInput")
    ident_r = nc.dram_tensor("ident_r", [P, P], F32R, kind="ExternalInput")
    ones_r = nc.dram_tensor("ones_r", [P, P], F32R, kind="ExternalInput")
    eps_b = nc.dram_tensor("eps_b", [P, 1], F32, kind="ExternalInput")
    kv0 = nc.dram_tensor("kv0", [HPC, HD, HD], F32, kind="ExternalInput")
    out = nc.dram_tensor("out", [SSH, H], F32, kind="ExternalOutput")
    DBG = bool(int(os.environ.get("KERNEL_DEBUG", "0")))
    if DBG:
        dbg_qkvT = nc.dram_tensor("dbg_qkvT", [MPC, SEQ], F32R, kind="ExternalOutput")
        dbg_gateT = nc.dram_tensor("dbg_gateT", [JPC, SEQ], F32, kind="ExternalOutput")
        dbg_hidT = nc.dram_tensor("dbg_hidT", [JPC, SEQ], F32, kind="ExternalOutput")
        dbg_y = nc.dram_tensor("dbg_y", [W, JPC, SSH], F32R, kind="ExternalOutput")
        dbg_a2a = nc.dram_tensor("dbg_a2a", [W, JPC, SSH], F32R, kind="ExternalOutput")
        dbg_ssq = nc.dram_tensor("dbg_ssq", [SEQ], F32, kind="ExternalOutput")
        dbg_rsq = nc.dram_tensor("dbg_rsq", [P, SSH // P], F32, kind="ExternalOutput")

    with tile.TileContext(nc) as tc:
        with tc.tile_pool(name="dram", bufs=1, space="DRAM") as dram, \
             tc.tile_pool(name="const", bufs=1) as const:
            # ---- DRAM temporaries ----
            qkvT_d0 = dram.tile([MPC // 2, SEQ], F32R)           # silu(qkv) heads 0-1, transposed
            qkvT_d1 = dram.tile([MPC // 2, SEQ], F32R)           # heads 2-3
            gateT_d = dram.tile([JPC, SEQ], F32)                 # sigmoid gate, transposed
            a2a_in = dram.tile([W, JPC, SSH], F32R)              # Y shards (j-major per shard)
            a2a_out = dram.tile([W, JPC, SSH], F32R)
            ssq_in = dram.tile([SEQ], F32)
            ssq_out = dram.tile([SSH], F32)

            # ---- constants in SBUF ----
            qdec_t = const.tile([P, HPC, BLOCK], F32)
            nc.sync.dma_start(qdec_t[:], qdec.ap()[:])
            kdec_t = const.tile([P, HPC, 2], F32)
            nc.sync.dma_start(kdec_t[:], kdec.ap()[:])
            maskT_t = const.tile([P, HPC, 2, BLOCK], F32)
            nc.sync.dma_start(maskT_t[:], maskT.ap()[:])
            blkdec_t = const.tile([P, HPC], F32)
            nc.sync.dma_start(blkdec_t[:], blkdec.ap()[:])
            ident_t = const.tile([P, P], F32R)
            nc.sync.dma_start(ident_t[:], ident_r.ap()[:])
            ones_t = const.tile([P, P], F32R)
            nc.sync.dma_start(ones_t[:], ones_r.ap()[:])
            eps_t = const.tile([P, 1], F32)
            nc.sync.dma_start(eps_t[:], eps_b.ap()[:])
            ssq_acc = const.tile([P, SB], F32)
            rsq = const.tile([P, SSH // P], F32)

            xT_v = xT.ap().rearrange("(ko p) s -> p ko s", p=P)
            wqkvT_v = wqkvT.ap().rearrange("(ko p) m -> p ko m", p=P)
            wgateT_v = wgateT.ap().rearrange("(ko p) m -> p ko m", p=P)
            qkvT_v0 = qkvT_d0.rearrange("(mo p) s -> p mo s", p=P)
            qkvT_v1 = qkvT_d1.rearrange("(mo p) s -> p mo s", p=P)
            gateT_v = gateT_d.rearrange("(go p) s -> p go s", p=P)

            # ================= phase AB: qkv + gate projections (bf16) ==========
            MG = MPC // 2   # 768 cols of wqkvT per group
            for grp in range(2):
                with tc.tile_pool(name=f"abw{grp}", bufs=1) as wp, \
                     tc.tile_pool(name=f"abx{grp}", bufs=2) as xp, \
                     tc.tile_pool(name=f"abo{grp}", bufs=4) as op, \
                     tc.tile_pool(name=f"abp{grp}", bufs=4, space="PSUM") as pp:
                    wq_t = wp.tile([P, KO, MG], BF16)
                    nc.sync.dma_start(wq_t[:], wqkvT_v[:, :, grp * MG:(grp + 1) * MG])
                    if grp == 0:
                        wg_t = wp.tile([P, KO, JPC], BF16)
                        nc.sync.dma_start(wg_t[:], wgateT_v[:])
                    for n in range(SEQ // 512):
                        x_t = xp.tile([P, KO, 512], BF16, tag="x")
                        nc.sync.dma_start(x_t[:], xT_v[:, :, n * 512:(n + 1) * 512])
                        for mm in range(MG // P):
                            ps = pp.tile([P, 512], F32, tag="ps")
                            for k in range(KO):
                                nc.tensor.matmul(ps[:], wq_t[:, k, mm * P:(mm + 1) * P],
                                                 x_t[:, k, :], start=(k == 0), stop=(k == KO - 1))
                            o_t = op.tile([P, 512], F32R, tag="o")
                            nc.scalar.activation(o_t[:], ps[:], AF.Silu)
                            qv = qkvT_v0 if grp == 0 else qkvT_v1
                            nc.sync.dma_start(qv[:, mm, n * 512:(n + 1) * 512], o_t[:])
                            if DBG:
                                m_global = grp * (MG // P) + mm
                                nc.sync.dma_start(
                                    dbg_qkvT.ap().rearrange("(mo p) s -> p mo s", p=P)[:, m_global, n * 512:(n + 1) * 512], o_t[:])
                        if grp == 0:
                            for gg in range(JPC // P):
                                ps = pp.tile([P, 512], F32, tag="ps")
                                for k in range(KO):
                                    nc.tensor.matmul(ps[:], wg_t[:, k, gg * P:(gg + 1) * P],
                                                     x_t[:, k, :], start=(k == 0), stop=(k == KO - 1))
                                g_t = op.tile([P, 512], F32, tag="g")
                                nc.scalar.activation(g_t[:], ps[:], AF.Sigmoid)
                                nc.sync.dma_start(gateT_v[:, gg, n * 512:(n + 1) * 512], g_t[:])
                                if DBG:
                                    nc.sync.dma_start(
                                        dbg_gateT.ap().rearrange("(go p) s -> p go s", p=P)[:, gg, n * 512:(n + 1) * 512], g_t[:])

            PHASES = os.environ.get("KERNEL_PHASES", "full")
            # ================= attention (fp32r) per head =======================
            if PHASES == "ab":
                return _finish(nc)
            with tc.tile_pool(name="atth", bufs=1) as ah, \
                 tc.tile_pool(name="atts", bufs=3) as asml, \
                 tc.tile_pool(name="attp", bufs=2, space="PSUM") as ap_:
                for hl in range(HPC):
                    qv = qkvT_v0 if hl < 2 else qkvT_v1
                    mo0 = 3 * (hl % 2)
                    qT_s = ah.tile([P, SEQ], F32R, tag="qT")
                    nc.sync.dma_start(qT_s[:], qv[:, mo0 + 0, :])
                    kT_s = ah.tile([P, SEQ], F32R, tag="kT")
                    nc.sync.dma_start(kT_s[:], qv[:, mo0 + 1, :])
                    vT_s = ah.tile([P, SEQ], F32R, tag="vT")
                    nc.sync.dma_start(vT_s[:], qv[:, mo0 + 2, :])
                    gate_h = ah.tile([P, SEQ], F32, tag="gate")
                    nc.sync.dma_start(gate_h[:], gateT_v[:, hl, :])
                    kv_state = ah.tile([P, HD], F32, tag="kv")
                    nc.sync.dma_start(kv_state[:], kv0.ap()[hl])

                    # k/v natural layouts via PE transpose; kdec folded into k copy
                    kd_nat = ah.tile([P, SB, HD], F32R, tag="kd")
                    v_nat = ah.tile([P, SB, HD], F32R, tag="vn")
                    for sb in range(SB):
                        pst = ap_.tile([P, P], F32R, tag="tr")
                        nc.tensor.transpose(pst[:], kT_s[:, sb * P:(sb + 1) * P], ident_t[:])
                        nc.scalar.activation(kd_nat[:, sb, :], pst[:], AF.Copy,
                                             scale=kdec_t[:, hl, sb % 2:sb % 2 + 1])
                        pst2 = ap_.tile([P, P], F32R, tag="tr")
                        nc.tensor.transpose(pst2[:], vT_s[:, sb * P:(sb + 1) * P], ident_t[:])
                        nc.any.tensor_copy(v_nat[:, sb, :], pst2[:])

                    hiddenT_h = ah.tile([P, SEQ], F32, tag="hid")

                    for b in range(NB):
                        msl = slice(b * BLOCK, (b + 1) * BLOCK)
                        qk_ps = []
                        for no in range(2):
                            qk = ap_.tile([P, BLOCK], F32, tag="qk")
                            nc.tensor.matmul(qk[:], kT_s[:, b * BLOCK + no * P: b * BLOCK + (no + 1) * P],
                                             qT_s[:, msl], start=True, stop=True)
                            qk_ps.append(qk)
                        qkTm = asml.tile([P, 2, BLOCK], F32R, tag="qkm")
                        for no in range(2):
                            nc.vector.tensor_mul(qkTm[:, no, :], qk_ps[no][:], maskT_t[:, hl, no, :])
                        qdT = asml.tile([P, BLOCK], F32R, tag="qdT")
                        nc.vector.tensor_mul(qdT[:], qT_s[:, msl], qdec_t[:, hl, :])
                        kv_r = asml.tile([P, HD], F32R, tag="kvr")
                        nc.scalar.activation(kv_r[:], kv_state[:], AF.Copy)

                        c_ps = ap_.tile([P, HD], F32, tag="C")
                        nc.tensor.matmul(c_ps[:], kd_nat[:, 2 * b, :], v_nat[:, 2 * b, :], start=True, stop=False)
                        nc.tensor.matmul(c_ps[:], kd_nat[:, 2 * b + 1, :], v_nat[:, 2 * b + 1, :], start=False, stop=True)

                        o_ps = ap_.tile([P, BLOCK], F32, tag="o")
                        nc.tensor.matmul(o_ps[:], v_nat[:, 2 * b, :], qkTm[:, 0, :], start=True, stop=False)
                        nc.tensor.matmul(o_ps[:], v_nat[:, 2 * b + 1, :], qkTm[:, 1, :], start=False, stop=False)
                        nc.tensor.matmul(o_ps[:], kv_r[:], qdT[:], start=False, stop=True)
                        nc.any.tensor_copy(hiddenT_h[:, msl], o_ps[:])
                        nc.vector.tensor_mul(kv_state[:], kv_state[:],
                                             blkdec_t[:, hl:hl + 1].to_broadcast([P, HD]))
                        nc.vector.tensor_add(kv_state[:], kv_state[:], c_ps[:])

                    # ssq partial + gated Y
                    sq_h = ah.tile([P, SEQ], F32R, tag="sq")
                    nc.vector.tensor_mul(sq_h[:], hiddenT_h[:], hiddenT_h[:])
                    for sc in range(SB):
                        sp = ap_.tile([P, P], F32, tag="C")
                        nc.tensor.matmul(sp[:], sq_h[:, sc * P:(sc + 1) * P], ones_t[:],
                                         start=True, stop=True)
                        if hl == 0:
                            nc.vector.tensor_copy(ssq_acc[:, sc:sc + 1], sp[:, 0:1])
                        else:
                            nc.vector.tensor_add(ssq_acc[:, sc:sc + 1], ssq_acc[:, sc:sc + 1], sp[:, 0:1])
                    if DBG:
                        nc.sync.dma_start(
                            dbg_hidT.ap().rearrange("(h p) s -> p h s", p=P)[:, hl, :], hiddenT_h[:])
                    y_h = ah.tile([P, SEQ], F32R, tag="y")
                    nc.vector.tensor_mul(y_h[:], hiddenT_h[:], gate_h[:])
                    for st in range(W):
                        nc.sync.dma_start(a2a_in[st, hl * P:(hl + 1) * P, :],
                                          y_h[:, st * SSH:(st + 1) * SSH])
                        if DBG:
                            nc.sync.dma_start(dbg_y.ap()[st, hl * P:(hl + 1) * P, :],
                                              y_h[:, st * SSH:(st + 1) * SSH])

            nc.sync.dma_start(ssq_in.rearrange("(a p) -> p a", p=P), ssq_acc[:])
            if DBG:
                nc.sync.dma_start(dbg_ssq.ap().rearrange("(a p) -> p a", p=P), ssq_acc[:])

            # ================= collectives ======================================
            if PHASES == "att":
                return _finish(nc)
            NOCOLL = bool(int(os.environ.get("KERNEL_NOCOLL", "0")))
            if NOCOLL:
                a2a_out = a2a_in
                ssq_out = ssq_in[:SSH]
            else:
                nc.gpsimd.collective_compute(
                    "AllToAll", ALU.bypass, replica_groups=[list(range(W))],
                    ins=[a2a_in.opt()], outs=[a2a_out.opt()])
                nc.gpsimd.collective_compute(
                    "ReduceScatter", ALU.add, replica_groups=[list(range(W))],
                    ins=[ssq_in.opt()], outs=[ssq_out.opt()])

            # rsqrt(var + eps)
            sq_raw = const.tile([P, SSH // P], F32)
            nc.sync.dma_start(sq_raw[:], ssq_out.rearrange("(i p) -> p i", p=P))
            t1 = const.tile([P, SSH // P], F32)
            nc.scalar.activation(t1[:], sq_raw[:], AF.Sqrt, bias=eps_t[:], scale=1.0 / INNER)
            nc.vector.reciprocal(rsq[:], t1[:])
            if DBG:
                nc.sync.dma_start(dbg_rsq.ap()[:], rsq[:])

            # ================= out projection (fp32r, seq-sharded) ==============
            with tc.tile_pool(name="oa", bufs=1) as oa, \
                 tc.tile_pool(name="ow", bufs=5) as ow, \
                 tc.tile_pool(name="oo", bufs=4) as oo, \
                 tc.tile_pool(name="op", bufs=4, space="PSUM") as opp:
                a2a_t = oa.tile([P, KO, SSH], F32R)
                nc.sync.dma_start(a2a_t[:], a2a_out.rearrange("r (jo p) s -> p (r jo) s", p=P))
                if DBG:
                    nc.sync.dma_start(dbg_a2a.ap().rearrange("r (jo p) s -> p (r jo) s", p=P), a2a_t[:])
                woutT_v = woutT.ap().rearrange("(ko p) o -> p ko o", p=P)
                out_v = out.ap().rearrange("(mo p) o -> p mo o", p=P)
                KC = KO // 4   # 8 k-subtiles per weight chunk
                for nt in range(H // 512):
                    osl = slice(nt * 512, (nt + 1) * 512)
                    w_ts = []
                    for q in range(4):
                        w_t = ow.tile([P, KC, 512], F32R, tag="w")
                        nc.sync.dma_start(w_t[:], woutT_v[:, q * KC:(q + 1) * KC, osl])
                        w_ts.append(w_t)
                    for mt in range(SSH // P):
                        ps = opp.tile([P, 512], F32, tag="po")
                        for kk in range(KO):
                            nc.tensor.matmul(ps[:], a2a_t[:, kk, mt * P:(mt + 1) * P],
                                             w_ts[kk // KC][:, kk % KC, :],
                                             start=(kk == 0), stop=(kk == KO - 1))
                        o_sb = oo.tile([P, 512], F32, tag="ot")
                        nc.vector.tensor_mul(o_sb[:], ps[:],
                                             rsq[:, mt:mt + 1].to_broadcast([P, 512]))
                        nc.sync.dma_start(out_v[:, mt, osl], o_sb[:])

    nc.compile()
    return nc


def _host_prep(inputs):
    x = np.asarray(inputs["x"], np.float32)
    w_qkv = np.asarray(inputs["w_qkv"], np.float32)
    w_gate = np.asarray(inputs["w_gate"], np.float32)
    w_out = np.asarray(inputs["w_out"], np.float32)
    norm_weight = np.asarray(inputs["norm_weight"], np.float32)
    kv_cache = np.asarray(inputs["kv_cache"], np.float32)
    slope = np.asarray(inputs["slope"], np.float32)

    bf = ml_dtypes.bfloat16
    xT_bf = np.ascontiguousarray(x.T).astype(bf)
    woutT = np.ascontiguousarray((w_out * norm_weight[None, :]).T).astype(np.float32)
    ident = np.eye(P, dtype=np.float32)
    ones = np.ones((P, P), np.float32)

    in_maps = []
    for c in range(W):
        sl = slope[c * HPC:(c + 1) * HPC]                     # [4]
        m0 = np.arange(BLOCK, dtype=np.float32)              # 0-based position in block
        # qdec[p, hl, m] = exp(-s*(m+1)) replicated over partitions
        qd = np.exp(-sl[:, None] * (m0[None, :] + 1.0))      # [4, 256]
        qdec_a = np.broadcast_to(qd[None], (P, HPC, BLOCK)).astype(np.float32).copy()
        # kdec[p, hl, no] = exp(-s*(BLOCK - (no*128+p+1)))
        n0 = (np.arange(2)[None, :] * P + np.arange(P)[:, None]).astype(np.float32)  # [128,2]
        kd = np.exp(-sl[None, None, :] * (BLOCK - (n0[:, :, None] + 1.0)))           # [128,2,4]
        kdec_a = np.ascontiguousarray(kd.transpose(0, 2, 1)).astype(np.float32)      # [128,4,2]
        # maskT[p, hl, no, m] = exp(-s*(m - n)) if m>=n else 0   (0-based n = no*128+p)
        nfull = n0[:, :, None]                                # [128,2,1]
        diff = m0[None, None, :] - nfull                      # [128,2,256]
        dif4 = diff[..., None]                                # [128,2,256,1]
        mask = np.where(dif4 >= 0,
                        np.exp(-sl[None, None, None, :] * np.maximum(dif4, 0.0)),
                        0.0)                                  # [128,2,256,4]
        maskT_a = np.ascontiguousarray(mask.transpose(0, 3, 1, 2)).astype(np.float32)        # [128,4,2,256]
        blkdec_a = np.broadcast_to(np.exp(-sl * BLOCK)[None], (P, HPC)).astype(np.float32).copy()

        in_maps.append({
            "xT": xT_bf,
            "wqkvT": np.ascontiguousarray(w_qkv[MPC * c:MPC * (c + 1)].T).astype(bf),
            "wgateT": np.ascontiguousarray(w_gate[JPC * c:JPC * (c + 1)].T).astype(bf),
            "woutT": woutT,
            "qdec": qdec_a,
            "kdec": kdec_a,
            "maskT": maskT_a,
            "blkdec": blkdec_a,
            "ident_r": ident,
            "ones_r": ones,
            "eps_b": np.full((P, 1), EPS, np.float32),
            "kv0": np.ascontiguousarray(kv_cache[HPC * c:HPC * (c + 1)]),
        })
    return in_maps


_CACHE = {}


def _get_program():
    if "nc" not in _CACHE:
        _CACHE["nc"] = _build_program()
    return _CACHE["nc"]


def kernel(**inputs):
    nc = _get_program()
    in_maps = _host_prep(inputs)
    trace = bool(int(os.environ.get("KERNEL_TRACE", "0")))
    res = run_bass_kernel_spmd(nc, in_maps, core_ids=list(range(W)), trace=trace)
    _CACHE["last_results"] = res
    out = np.concatenate([res.results[c]["out"] for c in range(W)], axis=0)
    return out.astype(np.float32)


# revision 19
# speedup vs baseline: 1.0377x; 1.0377x over previous
"""MiniMaxText01 linear attention (lightning attention) prefill layer on 8 TRN2 NeuronCores.

Strategy: tensor-parallel over heads (4 heads/core) for qkv+gate+attention;
AllToAll to switch to sequence-parallel for the output projection;
ReduceScatter for the RMSNorm variance. See spec sharding_hint.
"""
import os
import sys
import math

sys.path.insert(0, "/opt/trn_rl_repo")

import numpy as np
import ml_dtypes

import concourse.bass as bass
import concourse.mybir as mybir
import concourse.tile as tile
from concourse import bacc
from concourse.bass_utils import run_bass_kernel_spmd

# problem constants (hardcoded per contract)
H = 4096
INNER = 4096
NH = 32
HD = 128
SEQ = 4096
BLOCK = 256
EPS = 1e-5
P = 128
W = 8                    # cores
HPC = NH // W            # heads per core = 4
MPC = 3 * HD * HPC       # qkv rows per core = 1536
JPC = HD * HPC           # inner cols per core = 512
SSH = SEQ // W           # seq shard = 512
KO = H // P              # 32 k-subtiles
NB = SEQ // BLOCK        # 16 blocks
SB = SEQ // P            # 32 sub-blocks of 128

F32 = mybir.dt.float32
F32R = mybir.dt.float32r
BF16 = mybir.dt.bfloat16
AF = mybir.ActivationFunctionType
ALU = mybir.AluOpType


def _finish(nc):
    return nc


def _build_program():
    nc = bacc.Bacc("TRN2", target_bir_lowering=False, debug=False, num_devices=W)

    # ---- I/O ----
    xT = nc.dram_tensor("xT", [H, SEQ], BF16, kind="ExternalInput")
    wqkvT = nc.dram_tensor("wqkvT", [H, MPC], BF16, kind="ExternalInput")
    wgateT = nc.dram_tensor("wgateT", [H, JPC], BF16, kind="ExternalInput")
    woutT = nc.dram_tensor("woutT", [INNER, H], F32R, kind="ExternalInput")
    qdec = nc.dram_tensor("qdec", [P, HPC, BLOCK], F32, kind="ExternalInput")
    kdec = nc.dram_tensor("kdec", [P, HPC, 2], F32, kind="ExternalInput")
    maskT = nc.dram_tensor("maskT", [P, HPC, 2, BLOCK], F32, kind="ExternalInput")
    blkdec = nc.dram_tensor("blkdec", [P, HPC], F32, kind="ExternalInput")
    ident_r = nc.dram_tensor("ident_r", [P, P], F32R, kind="ExternalInput")
    ones_r = nc.dram_tensor("ones_r", [P, P], F32R, kind="ExternalInput")
    eps_b = nc.dram_tensor("eps_b", [P, 1], F32, kind="ExternalInput")
    kv0 = nc.dram_tensor("kv0", [HPC, HD, HD], F32, kind="ExternalInput")
    out = nc.dram_tensor("out", [SSH, H], F32, kind="ExternalOutput")
    DBG = bool(int(os.environ.get("KERNEL_DEBUG", "0")))
    if DBG:
        dbg_qkvT = nc.dram_tensor("dbg_qkvT", [MPC, SEQ], F32R, kind="ExternalOutput")
        dbg_gateT = nc.dram_tensor("dbg_gateT", [JPC, SEQ], F32, kind="ExternalOutput")
        dbg_hidT = nc.dram_tensor("dbg_hidT", [JPC, SEQ], F32, kind="ExternalOutput")
        dbg_y = nc.dram_tensor("dbg_y", [W, JPC, SSH], F32R, kind="ExternalOutput")
        dbg_a2a = nc.dram_tensor("dbg_a2a", [W, JPC, SSH], F32R, kind="ExternalOutput")
        dbg_ssq = nc.dram_tensor("dbg_ssq", [SEQ], F32, kind="ExternalOutput")
        dbg_rsq = nc.dram_tensor("dbg_rsq", [P, SSH // P], F32, kind="ExternalOutput")

    with tile.TileContext(nc) as tc:
        with tc.tile_pool(name="dram", bufs=1, space="DRAM") as dram, \
             tc.tile_pool(name="const", bufs=1) as const:
            # ---- DRAM temporaries ----
            qkvT_d0 = dram.tile([MPC // 2, SEQ], F32R)           # silu(qkv) heads 0-1, transposed
            qkvT_d1 = dram.tile([MPC // 2, SEQ], F32R)           # heads 2-3
            gateT_d = dram.tile([JPC, SEQ], F32)                 # sigmoid gate, transposed
            a2a_in = dram.tile([W, JPC, SSH], F32R)              # Y shards (j-major per shard)
            a2a_out = dram.tile([W, JPC, SSH], F32R)
            ssq_in = dram.tile([SEQ], F32)
            ssq_out = dram.tile([SSH], F32)

            # ---- constants in SBUF ----
            qdec_t = const.tile([P, HPC, BLOCK], F32)
            nc.sync.dma_start(qdec_t[:], qdec.ap()[:])
            kdec_t = const.tile([P, HPC, 2], F32)
            nc.sync.dma_start(kdec_t[:], kdec.ap()[:])
            maskT_t = const.tile([P, HPC, 2, BLOCK], F32)
            nc.sync.dma_start(maskT_t[:], maskT.ap()[:])
            blkdec_t = const.tile([P, HPC], F32)
            nc.sync.dma_start(blkdec_t[:], blkdec.ap()[:])
            ident_t = const.tile([P, P], F32R)
            nc.sync.dma_start(ident_t[:], ident_r.ap()[:])
            ones_t = const.tile([P, P], F32R)
            nc.sync.dma_start(ones_t[:], ones_r.ap()[:])
            eps_t = const.tile([P, 1], F32)
            nc.sync.dma_start(eps_t[:], eps_b.ap()[:])
            ssq_acc = const.tile([P, SB], F32)
            rsq = const.tile([P, SSH // P], F32)

            xT_v = xT.ap().rearrange("(ko p) s -> p ko s", p=P)
            wqkvT_v = wqkvT.ap().rearrange("(ko p) m -> p ko m", p=P)
            wgateT_v = wgateT.ap().rearrange("(ko p) m -> p ko m", p=P)
            qkvT_v0 = qkvT_d0.rearrange("(mo p) s -> p mo s", p=P)
            qkvT_v1 = qkvT_d1.rearrange("(mo p) s -> p mo s", p=P)
            gateT_v = gateT_d.rearrange("(go p) s -> p go s", p=P)

            # ================= phase AB: qkv + gate projections (bf16) ==========
            MG = MPC // 2   # 768 cols of wqkvT per group
            KC = 8          # k-subtiles per chunk (4 chunks cover K)
            NKC = KO // KC
            with tc.tile_pool(name="abw", bufs=NKC + 1) as wp, \
                 tc.tile_pool(name="abg", bufs=NKC) as gp, \
                 tc.tile_pool(name="abx", bufs=NKC + 2) as xp, \
                 tc.tile_pool(name="abo", bufs=4) as op, \
                 tc.tile_pool(name="abp", bufs=4, space="PSUM") as pp:
                wg_c = []
                for kc in range(NKC):
                    wg_t = gp.tile([P, KC, JPC], BF16, tag="wg", name=f"wg{kc}")
                    nc.sync.dma_start(wg_t[:], wgateT_v[:, kc * KC:(kc + 1) * KC, :])
                    wg_c.append(wg_t)
                for grp in range(2):
                    wq_c = []
                    for kc in range(NKC):
                        wq_t = wp.tile([P, KC, MG], BF16, tag="wq", name=f"wq{grp}_{kc}")
                        nc.sync.dma_start(wq_t[:], wqkvT_v[:, kc * KC:(kc + 1) * KC,
                                                           grp * MG:(grp + 1) * MG])
                        wq_c.append(wq_t)
                    for n in range(SEQ // 512):
                        x_c = []
                        for kc in range(NKC):
                            x_t = xp.tile([P, KC, 512], BF16, tag="x", name=f"x{grp}_{n}_{kc}")
                            nc.sync.dma_start(x_t[:], xT_v[:, kc * KC:(kc + 1) * KC,
                                                           n * 512:(n + 1) * 512])
                            x_c.append(x_t)
                        for mm in range(MG // P):
                            ps = pp.tile([P, 512], F32, tag="ps")
                            for k in range(KO):
                                nc.tensor.matmul(ps[:], wq_c[k // KC][:, k % KC, mm * P:(mm + 1) * P],
                                                 x_c[k // KC][:, k % KC, :],
                                                 start=(k == 0), stop=(k == KO - 1))
                            o_t = op.tile([P, 512], F32R, tag="o")
                            nc.scalar.activation(o_t[:], ps[:], AF.Silu)
                            qv = qkvT_v0 if grp == 0 else qkvT_v1
                            nc.sync.dma_start(qv[:, mm, n * 512:(n + 1) * 512], o_t[:])
                            if DBG:
                                m_global = grp * (MG // P) + mm
                                nc.sync.dma_start(
                                    dbg_qkvT.ap().rearrange("(mo p) s -> p mo s", p=P)[:, m_global, n * 512:(n + 1) * 512], o_t[:])
                        if grp == 0:
                            for gg in range(JPC // P):
                                ps = pp.tile([P, 512], F32, tag="ps")
                                for k in range(KO):
                                    nc.tensor.matmul(ps[:], wg_c[k // KC][:, k % KC, gg * P:(gg + 1) * P],
                                                     x_c[k // KC][:, k % KC, :],
                                                     start=(k == 0), stop=(k == KO - 1))
                                g_t = op.tile([P, 512], F32, tag="g")
                                nc.scalar.activation(g_t[:], ps[:], AF.Sigmoid)
                                nc.sync.dma_start(gateT_v[:, gg, n * 512:(n + 1) * 512], g_t[:])
                                if DBG:
                                    nc.sync.dma_start(
                                        dbg_gateT.ap().rearrange("(go p) s -> p go s", p=P)[:, gg, n * 512:(n + 1) * 512], g_t[:])

            PHASES = os.environ.get("KERNEL_PHASES", "full")
            # ================= attention (fp32r) per head =======================
            if PHASES == "ab":
                return _finish(nc)
            with tc.tile_pool(name="atth", bufs=1) as ah, \
                 tc.tile_pool(name="atts", bufs=3) as asml, \
                 tc.tile_pool(name="attp", bufs=2, space="PSUM") as ap_:
                for hl in range(HPC):
                    qv = qkvT_v0 if hl < 2 else qkvT_v1
                    mo0 = 3 * (hl % 2)
                    qT_s = ah.tile([P, SEQ], F32R, tag="qT")
                    nc.sync.dma_start(qT_s[:], qv[:, mo0 + 0, :])
                    kT_s = ah.tile([P, SEQ], F32R, tag="kT")
                    nc.sync.dma_start(kT_s[:], qv[:, mo0 + 1, :])
                    vT_s = ah.tile([P, SEQ], F32R, tag="vT")
                    nc.sync.dma_start(vT_s[:], qv[:, mo0 + 2, :])
                    gate_h = ah.tile([P, SEQ], F32, tag="gate")
                    nc.sync.dma_start(gate_h[:], gateT_v[:, hl, :])
                    kv_state = ah.tile([P, HD], F32, tag="kv")
                    nc.sync.dma_start(kv_state[:], kv0.ap()[hl])

                    # k/v natural layouts via PE transpose; kdec folded into k copy
                    kd_nat = ah.tile([P, SB, HD], F32R, tag="kd")
                    v_nat = ah.tile([P, SB, HD], F32R, tag="vn")
                    for sb in range(SB):
                        pst = ap_.tile([P, P], F32R, tag="tr")
                        nc.tensor.transpose(pst[:], kT_s[:, sb * P:(sb + 1) * P], ident_t[:])
                        nc.scalar.activation(kd_nat[:, sb, :], pst[:], AF.Copy,
                                             scale=kdec_t[:, hl, sb % 2:sb % 2 + 1])
                        pst2 = ap_.tile([P, P], F32R, tag="tr")
                        nc.tensor.transpose(pst2[:], vT_s[:, sb * P:(sb + 1) * P], ident_t[:])
                        nc.any.tensor_copy(v_nat[:, sb, :], pst2[:])

                    hiddenT_h = ah.tile([P, SEQ], F32, tag="hid")

                    for b in range(NB):
                        msl = slice(b * BLOCK, (b + 1) * BLOCK)
                        qk_ps = []
                        for no in range(2):
                            qk = ap_.tile([P, BLOCK], F32, tag="qk")
                            nc.tensor.matmul(qk[:], kT_s[:, b * BLOCK + no * P: b * BLOCK + (no + 1) * P],
                                             qT_s[:, msl], start=True, stop=True)
                            qk_ps.append(qk)
                        qkTm = asml.tile([P, 2, BLOCK], F32R, tag="qkm")
                        for no in range(2):
                            nc.vector.tensor_mul(qkTm[:, no, :], qk_ps[no][:], maskT_t[:, hl, no, :])
                        qdT = asml.tile([P, BLOCK], F32R, tag="qdT")
                        nc.vector.tensor_mul(qdT[:], qT_s[:, msl], qdec_t[:, hl, :])
                        kv_r = asml.tile([P, HD], F32R, tag="kvr")
                        nc.scalar.activation(kv_r[:], kv_state[:], AF.Copy)

                        c_ps = ap_.tile([P, HD], F32, tag="C")
                        nc.tensor.matmul(c_ps[:], kd_nat[:, 2 * b, :], v_nat[:, 2 * b, :], start=True, stop=False)
                        nc.tensor.matmul(c_ps[:], kd_nat[:, 2 * b + 1, :], v_nat[:, 2 * b + 1, :], start=False, stop=True)

                        o_ps = ap_.tile([P, BLOCK], F32, tag="o")
                        nc.tensor.matmul(o_ps[:], v_nat[:, 2 * b, :], qkTm[:, 0, :], start=True, stop=False)
                        nc.tensor.matmul(o_ps[:], v_nat[:, 2 * b + 1, :], qkTm[:, 1, :], start=False, stop=False)
                        nc.tensor.matmul(o_ps[:], kv_r[:], qdT[:], start=False, stop=True)
                        nc.any.tensor_copy(hiddenT_h[:, msl], o_ps[:])
                        nc.vector.tensor_mul(kv_state[:], kv_state[:],
                                             blkdec_t[:, hl:hl + 1].to_broadcast([P, HD]))
                        nc.vector.tensor_add(kv_state[:], kv_state[:], c_ps[:])

                    # ssq partial + gated Y
                    sq_h = ah.tile([P, SEQ], F32R, tag="sq")
                    nc.vector.tensor_mul(sq_h[:], hiddenT_h[:], hiddenT_h[:])
                    for sc in range(SB):
                        sp = ap_.tile([P, P], F32, tag="C")
                        nc.tensor.matmul(sp[:], sq_h[:, sc * P:(sc + 1) * P], ones_t[:],
                                         start=True, stop=True)
                        if hl == 0:
                            nc.vector.tensor_copy(ssq_acc[:, sc:sc + 1], sp[:, 0:1])
                        else:
                            nc.vector.tensor_add(ssq_acc[:, sc:sc + 1], ssq_acc[:, sc:sc + 1], sp[:, 0:1])
                    if DBG:
                        nc.sync.dma_start(
                            dbg_hidT.ap().rearrange("(h p) s -> p h s", p=P)[:, hl, :], hiddenT_h[:])
                    y_h = ah.tile([P, SEQ], F32R, tag="y")
                    nc.vector.tensor_mul(y_h[:], hiddenT_h[:], gate_h[:])
                    for st in range(W):
                        nc.sync.dma_start(a2a_in[st, hl * P:(hl + 1) * P, :],
                                          y_h[:, st * SSH:(st + 1) * SSH])
                        if DBG:
                            nc.sync.dma_start(dbg_y.ap()[st, hl * P:(hl + 1) * P, :],
                                              y_h[:, st * SSH:(st + 1) * SSH])

            nc.sync.dma_start(ssq_in.rearrange("(a p) -> p a", p=P), ssq_acc[:])
            if DBG:
                nc.sync.dma_start(dbg_ssq.ap().rearrange("(a p) -> p a", p=P), ssq_acc[:])

            # ================= collectives ======================================
            if PHASES == "att":
                return _finish(nc)
            NOCOLL = bool(int(os.environ.get("KERNEL_NOCOLL", "0")))
            if NOCOLL:
                a2a_out = a2a_in
                ssq_out = ssq_in[:SSH]
            else:
                nc.gpsimd.collective_compute(
                    "AllToAll", ALU.bypass, replica_groups=[list(range(W))],
                    ins=[a2a_in.opt()], outs=[a2a_out.opt()])
                nc.gpsimd.collective_compute(
                    "ReduceScatter", ALU.add, replica_groups=[list(range(W))],
                    ins=[ssq_in.opt()], outs=[ssq_out.opt()])

            # rsqrt(var + eps)
            sq_raw = const.tile([P, SSH // P], F32)
            nc.sync.dma_start(sq_raw[:], ssq_out.rearrange("(i p) -> p i", p=P))
            t1 = const.tile([P, SSH // P], F32)
            nc.scalar.activation(t1[:], sq_raw[:], AF.Sqrt, bias=eps_t[:], scale=1.0 / INNER)
            nc.vector.reciprocal(rsq[:], t1[:])
            if DBG:
                nc.sync.dma_start(dbg_rsq.ap()[:], rsq[:])

            # ================= out projection (fp32r, seq-sharded) ==============
            with tc.tile_pool(name="oa", bufs=1) as oa, \
                 tc.tile_pool(name="ow", bufs=5) as ow, \
                 tc.tile_pool(name="oo", bufs=4) as oo, \
                 tc.tile_pool(name="op", bufs=4, space="PSUM") as opp:
                a2a_t = oa.tile([P, KO, SSH], F32R)
                a2a_v = a2a_out.rearrange("r (jo p) s -> p (r jo) s", p=P)
                for q in range(4):
                    nc.sync.dma_start(a2a_t[:, q * (KO // 4):(q + 1) * (KO // 4), :],
                                      a2a_v[:, q * (KO // 4):(q + 1) * (KO // 4), :])
                if DBG:
                    nc.sync.dma_start(dbg_a2a.ap().rearrange("r (jo p) s -> p (r jo) s", p=P), a2a_t[:])
                woutT_v = woutT.ap().rearrange("(ko p) o -> p ko o", p=P)
                out_v = out.ap().rearrange("(mo p) o -> p mo o", p=P)
                KC = KO // 4   # 8 k-subtiles per weight chunk
                for nt in range(H // 512):
                    osl = slice(nt * 512, (nt + 1) * 512)
                    w_ts = []
                    for q in range(4):
                        w_t = ow.tile([P, KC, 512], F32R, tag="w")
                        nc.sync.dma_start(w_t[:], woutT_v[:, q * KC:(q + 1) * KC, osl])
                        w_ts.append(w_t)
                    for mt in range(SSH // P):
                        ps = opp.tile([P, 512], F32, tag="po")
                        for kk in range(KO):
                            nc.tensor.matmul(ps[:], a2a_t[:, kk, mt * P:(mt + 1) * P],
                                             w_ts[kk // KC][:, kk % KC, :],
                                             start=(kk == 0), stop=(kk == KO - 1))
                        o_sb = oo.tile([P, 512], F32, tag="ot")
                        nc.vector.tensor_mul(o_sb[:], ps[:],
                                             rsq[:, mt:mt + 1].to_broadcast([P, 512]))
                        nc.sync.dma_start(out_v[:, mt, osl], o_sb[:])

    nc.compile()
    return nc


def _host_prep(inputs):
    x = np.asarray(inputs["x"], np.float32)
    w_qkv = np.asarray(inputs["w_qkv"], np.float32)
    w_gate = np.asarray(inputs["w_gate"], np.float32)
    w_out = np.asarray(inputs["w_out"], np.float32)
    norm_weight = np.asarray(inputs["norm_weight"], np.float32)
    kv_cache = np.asarray(inputs["kv_cache"], np.float32)
    slope = np.asarray(inputs["slope"], np.float32)

    bf = ml_dtypes.bfloat16
    xT_bf = np.ascontiguousarray(x.T).astype(bf)
    woutT = np.ascontiguousarray((w_out * norm_weight[None, :]).T).astype(np.float32)
    ident = np.eye(P, dtype=np.float32)
    ones = np.ones((P, P), np.float32)

    in_maps = []
    for c in range(W):
        sl = slope[c * HPC:(c + 1) * HPC]                     # [4]
        m0 = np.arange(BLOCK, dtype=np.float32)              # 0-based position in block
        # qdec[p, hl, m] = exp(-s*(m+1)) replicated over partitions
        qd = np.exp(-sl[:, None] * (m0[None, :] + 1.0))      # [4, 256]
        qdec_a = np.broadcast_to(qd[None], (P, HPC, BLOCK)).astype(np.float32).copy()
        # kdec[p, hl, no] = exp(-s*(BLOCK - (no*128+p+1)))
        n0 = (np.arange(2)[None, :] * P + np.arange(P)[:, None]).astype(np.float32)  # [128,2]
        kd = np.exp(-sl[None, None, :] * (BLOCK - (n0[:, :, None] + 1.0)))           # [128,2,4]
        kdec_a = np.ascontiguousarray(kd.transpose(0, 2, 1)).astype(np.float32)      # [128,4,2]
        # maskT[p, hl, no, m] = exp(-s*(m - n)) if m>=n else 0   (0-based n = no*128+p)
        nfull = n0[:, :, None]                                # [128,2,1]
        diff = m0[None, None, :] - nfull                      # [128,2,256]
        dif4 = diff[..., None]                                # [128,2,256,1]
        mask = np.where(dif4 >= 0,
                        np.exp(-sl[None, None, None, :] * np.maximum(dif4, 0.0)),
                        0.0)                                  # [128,2,256,4]
        maskT_a = np.ascontiguousarray(mask.transpose(0, 3, 1, 2)).astype(np.float32)        # [128,4,2,256]
        blkdec_a = np.broadcast_to(np.exp(-sl * BLOCK)[None], (P, HPC)).astype(np.float32).copy()

        in_maps.append({
            "xT": xT_bf,
            "wqkvT": np.ascontiguousarray(w_qkv[MPC * c:MPC * (c + 1)].T).astype(bf),
            "wgateT": np.ascontiguousarray(w_gate[JPC * c:JPC * (c + 1)].T).astype(bf),
            "woutT": woutT,
            "qdec": qdec_a,
            "kdec": kdec_a,
            "maskT": maskT_a,
            "blkdec": blkdec_a,
            "ident_r": ident,
            "ones_r": ones,
            "eps_b": np.full((P, 1), EPS, np.float32),
            "kv0": np.ascontiguousarray(kv_cache[HPC * c:HPC * (c + 1)]),
        })
    return in_maps


_CACHE = {}


def _get_program():
    if "nc" not in _CACHE:
        _CACHE["nc"] = _build_program()
    return _CACHE["nc"]


def kernel(**inputs):
    nc = _get_program()
    in_maps = _host_prep(inputs)
    trace = bool(int(os.environ.get("KERNEL_TRACE", "0")))
    res = run_bass_kernel_spmd(nc, in_maps, core_ids=list(range(W)), trace=trace)
    _CACHE["last_results"] = res
    out = np.concatenate([res.results[c]["out"] for c in range(W)], axis=0)
    return out.astype(np.float32)


# revision 20
# speedup vs baseline: 1.0466x; 1.0086x over previous
"""MiniMaxText01 linear attention (lightning attention) prefill layer on 8 TRN2 NeuronCores.

Strategy: tensor-parallel over heads (4 heads/core) for qkv+gate+attention;
AllToAll to switch to sequence-parallel for the output projection;
ReduceScatter for the RMSNorm variance. See spec sharding_hint.
"""
import os
import sys
import math

sys.path.insert(0, "/opt/trn_rl_repo")

import numpy as np
import ml_dtypes

import concourse.bass as bass
import concourse.mybir as mybir
import concourse.tile as tile
from concourse import bacc
from concourse.bass_utils import run_bass_kernel_spmd

# problem constants (hardcoded per contract)
H = 4096
INNER = 4096
NH = 32
HD = 128
SEQ = 4096
BLOCK = 256
EPS = 1e-5
P = 128
W = 8                    # cores
HPC = NH // W            # heads per core = 4
MPC = 3 * HD * HPC       # qkv rows per core = 1536
JPC = HD * HPC           # inner cols per core = 512
SSH = SEQ // W           # seq shard = 512
KO = H // P              # 32 k-subtiles
NB = SEQ // BLOCK        # 16 blocks
SB = SEQ // P            # 32 sub-blocks of 128

F32 = mybir.dt.float32
F32R = mybir.dt.float32r
BF16 = mybir.dt.bfloat16
AF = mybir.ActivationFunctionType
ALU = mybir.AluOpType


def _finish(nc):
    return nc


def _build_program():
    nc = bacc.Bacc("TRN2", target_bir_lowering=False, debug=False, num_devices=W)

    # ---- I/O ----
    xT = nc.dram_tensor("xT", [H, SEQ], BF16, kind="ExternalInput")
    wqkvT = nc.dram_tensor("wqkvT", [H, MPC], BF16, kind="ExternalInput")
    wgateT = nc.dram_tensor("wgateT", [H, JPC], BF16, kind="ExternalInput")
    woutT = nc.dram_tensor("woutT", [INNER, H], F32R, kind="ExternalInput")
    qdec = nc.dram_tensor("qdec", [P, HPC, BLOCK], F32, kind="ExternalInput")
    kdec = nc.dram_tensor("kdec", [P, HPC, 2], F32, kind="ExternalInput")
    maskT = nc.dram_tensor("maskT", [P, HPC, 2, BLOCK], F32, kind="ExternalInput")
    blkdec = nc.dram_tensor("blkdec", [P, HPC], F32, kind="ExternalInput")
    ident_r = nc.dram_tensor("ident_r", [P, P], F32R, kind="ExternalInput")
    ones_r = nc.dram_tensor("ones_r", [P, P], F32R, kind="ExternalInput")
    eps_b = nc.dram_tensor("eps_b", [P, 1], F32, kind="ExternalInput")
    kv0 = nc.dram_tensor("kv0", [HPC, HD, HD], F32, kind="ExternalInput")
    out = nc.dram_tensor("out", [SSH, H], F32, kind="ExternalOutput")
    DBG = bool(int(os.environ.get("KERNEL_DEBUG", "0")))
    if DBG:
        dbg_qkvT = nc.dram_tensor("dbg_qkvT", [MPC, SEQ], F32R, kind="ExternalOutput")
        dbg_gateT = nc.dram_tensor("dbg_gateT", [JPC, SEQ], F32, kind="ExternalOutput")
        dbg_hidT = nc.dram_tensor("dbg_hidT", [JPC, SEQ], F32, kind="ExternalOutput")
        dbg_y = nc.dram_tensor("dbg_y", [W, JPC, SSH], F32R, kind="ExternalOutput")
        dbg_a2a = nc.dram_tensor("dbg_a2a", [W, JPC, SSH], F32R, kind="ExternalOutput")
        dbg_ssq = nc.dram_tensor("dbg_ssq", [SEQ], F32, kind="ExternalOutput")
        dbg_rsq = nc.dram_tensor("dbg_rsq", [P, SSH // P], F32, kind="ExternalOutput")

    with tile.TileContext(nc) as tc:
        with tc.tile_pool(name="dram", bufs=1, space="DRAM") as dram, \
             tc.tile_pool(name="const", bufs=1) as const:
            # ---- DRAM temporaries ----
            qkvT_d0 = dram.tile([MPC // 2, SEQ], F32R)           # silu(qkv) heads 0-1, transposed
            qkvT_d1 = dram.tile([MPC // 2, SEQ], F32R)           # heads 2-3
            gateT_d = dram.tile([JPC, SEQ], F32)                 # sigmoid gate, transposed
            a2a_in = dram.tile([W, JPC, SSH], F32R)              # Y shards (j-major per shard)
            a2a_out = dram.tile([W, JPC, SSH], F32R)
            ssq_in = dram.tile([SEQ], F32)
            ssq_out = dram.tile([SSH], F32)

            # ---- constants in SBUF ----
            qdec_t = const.tile([P, HPC, BLOCK], F32)
            nc.sync.dma_start(qdec_t[:], qdec.ap()[:])
            kdec_t = const.tile([P, HPC, 2], F32)
            nc.sync.dma_start(kdec_t[:], kdec.ap()[:])
            maskT_t = const.tile([P, HPC, 2, BLOCK], F32)
            nc.sync.dma_start(maskT_t[:], maskT.ap()[:])
            blkdec_t = const.tile([P, HPC], F32)
            nc.sync.dma_start(blkdec_t[:], blkdec.ap()[:])
            ident_t = const.tile([P, P], F32R)
            nc.sync.dma_start(ident_t[:], ident_r.ap()[:])
            ones_t = const.tile([P, P], F32R)
            nc.sync.dma_start(ones_t[:], ones_r.ap()[:])
            eps_t = const.tile([P, 1], F32)
            nc.sync.dma_start(eps_t[:], eps_b.ap()[:])
            ssq_acc = const.tile([P, SB], F32)
            rsq = const.tile([P, SSH // P], F32)

            xT_v = xT.ap().rearrange("(ko p) s -> p ko s", p=P)
            wqkvT_v = wqkvT.ap().rearrange("(ko p) m -> p ko m", p=P)
            wgateT_v = wgateT.ap().rearrange("(ko p) m -> p ko m", p=P)
            qkvT_v0 = qkvT_d0.rearrange("(mo p) s -> p mo s", p=P)
            qkvT_v1 = qkvT_d1.rearrange("(mo p) s -> p mo s", p=P)
            gateT_v = gateT_d.rearrange("(go p) s -> p go s", p=P)

            # ================= phase AB: qkv + gate projections (bf16) ==========
            MG = MPC // 2   # 768 cols of wqkvT per group
            KC = 8          # k-subtiles per chunk (4 chunks cover K)
            NKC = KO // KC
            with tc.tile_pool(name="abw", bufs=NKC + 1) as wp, \
                 tc.tile_pool(name="abg", bufs=NKC) as gp, \
                 tc.tile_pool(name="abx", bufs=NKC + 2) as xp, \
                 tc.tile_pool(name="abo", bufs=4) as op, \
                 tc.tile_pool(name="abp", bufs=4, space="PSUM") as pp:
                wg_c = []
                for kc in range(NKC):
                    wg_t = gp.tile([P, KC, JPC], BF16, tag="wg", name=f"wg{kc}")
                    nc.sync.dma_start(wg_t[:], wgateT_v[:, kc * KC:(kc + 1) * KC, :])
                    wg_c.append(wg_t)
                for grp in range(2):
                    wq_c = []
                    for kc in range(NKC):
                        wq_t = wp.tile([P, KC, MG], BF16, tag="wq", name=f"wq{grp}_{kc}")
                        nc.sync.dma_start(wq_t[:], wqkvT_v[:, kc * KC:(kc + 1) * KC,
                                                           grp * MG:(grp + 1) * MG])
                        wq_c.append(wq_t)
                    for n in range(SEQ // 512):
                        x_c = []
                        for kc in range(NKC):
                            x_t = xp.tile([P, KC, 512], BF16, tag="x", name=f"x{grp}_{n}_{kc}")
                            nc.sync.dma_start(x_t[:], xT_v[:, kc * KC:(kc + 1) * KC,
                                                           n * 512:(n + 1) * 512])
                            x_c.append(x_t)
                        for mm in range(MG // P):
                            ps = pp.tile([P, 512], F32, tag="ps")
                            for k in range(KO):
                                nc.tensor.matmul(ps[:], wq_c[k // KC][:, k % KC, mm * P:(mm + 1) * P],
                                                 x_c[k // KC][:, k % KC, :],
                                                 start=(k == 0), stop=(k == KO - 1))
                            o_t = op.tile([P, 512], F32R, tag="o")
                            nc.scalar.activation(o_t[:], ps[:], AF.Silu)
                            qv = qkvT_v0 if grp == 0 else qkvT_v1
                            nc.sync.dma_start(qv[:, mm, n * 512:(n + 1) * 512], o_t[:])
                            if DBG:
                                m_global = grp * (MG // P) + mm
                                nc.sync.dma_start(
                                    dbg_qkvT.ap().rearrange("(mo p) s -> p mo s", p=P)[:, m_global, n * 512:(n + 1) * 512], o_t[:])
                        if grp == 0:
                            for gg in range(JPC // P):
                                ps = pp.tile([P, 512], F32, tag="ps")
                                for k in range(KO):
                                    nc.tensor.matmul(ps[:], wg_c[k // KC][:, k % KC, gg * P:(gg + 1) * P],
                                                     x_c[k // KC][:, k % KC, :],
                                                     start=(k == 0), stop=(k == KO - 1))
                                g_t = op.tile([P, 512], F32, tag="g")
                                nc.scalar.activation(g_t[:], ps[:], AF.Sigmoid)
                                nc.sync.dma_start(gateT_v[:, gg, n * 512:(n + 1) * 512], g_t[:])
                                if DBG:
                                    nc.sync.dma_start(
                                        dbg_gateT.ap().rearrange("(go p) s -> p go s", p=P)[:, gg, n * 512:(n + 1) * 512], g_t[:])

            PHASES = os.environ.get("KERNEL_PHASES", "full")
            # ================= attention (fp32r) per head =======================
            if PHASES == "ab":
                return _finish(nc)
            with tc.tile_pool(name="atth", bufs=1) as ah, \
                 tc.tile_pool(name="atts", bufs=3) as asml, \
                 tc.tile_pool(name="attp", bufs=2, space="PSUM") as ap_:
                for hl in range(HPC):
                    qv = qkvT_v0 if hl < 2 else qkvT_v1
                    mo0 = 3 * (hl % 2)
                    qT_s = ah.tile([P, SEQ], F32R, tag="qT")
                    nc.sync.dma_start(qT_s[:], qv[:, mo0 + 0, :])
                    kT_s = ah.tile([P, SEQ], F32R, tag="kT")
                    nc.sync.dma_start(kT_s[:], qv[:, mo0 + 1, :])
                    vT_s = ah.tile([P, SEQ], F32R, tag="vT")
                    nc.sync.dma_start(vT_s[:], qv[:, mo0 + 2, :])
                    gate_h = ah.tile([P, SEQ], F32, tag="gate")
                    nc.sync.dma_start(gate_h[:], gateT_v[:, hl, :])
                    kv_state = ah.tile([P, HD], F32, tag="kv")
                    nc.sync.dma_start(kv_state[:], kv0.ap()[hl])

                    # k/v natural layouts via PE transpose; kdec folded into k copy
                    kd_nat = ah.tile([P, SB, HD], F32R, tag="kd")
                    v_nat = ah.tile([P, SB, HD], F32R, tag="vn")
                    for sb in range(SB):
                        pst = ap_.tile([P, P], F32R, tag="tr")
                        nc.tensor.transpose(pst[:], kT_s[:, sb * P:(sb + 1) * P], ident_t[:])
                        nc.scalar.activation(kd_nat[:, sb, :], pst[:], AF.Copy,
                                             scale=kdec_t[:, hl, sb % 2:sb % 2 + 1])
                        pst2 = ap_.tile([P, P], F32R, tag="tr")
                        nc.tensor.transpose(pst2[:], vT_s[:, sb * P:(sb + 1) * P], ident_t[:])
                        nc.any.tensor_copy(v_nat[:, sb, :], pst2[:])

                    hiddenT_h = ah.tile([P, SEQ], F32, tag="hid")

                    # kv prefix scan: kv_all[:, b] = state BEFORE block b. C matmuls
                    # are independent; only the cheap DVE scan is serial, so the
                    # o-matmuls below never wait on the recurrence.
                    kv_all = ah.tile([P, NB, HD], F32R, tag="kva")
                    nc.scalar.activation(kv_all[:, 0, :], kv_state[:], AF.Copy)
                    for b in range(NB - 1):
                        c_ps = ap_.tile([P, HD], F32, tag="C")
                        nc.tensor.matmul(c_ps[:], kd_nat[:, 2 * b, :], v_nat[:, 2 * b, :], start=True, stop=False)
                        nc.tensor.matmul(c_ps[:], kd_nat[:, 2 * b + 1, :], v_nat[:, 2 * b + 1, :], start=False, stop=True)
                        nc.vector.tensor_mul(kv_all[:, b + 1, :], kv_all[:, b, :],
                                             blkdec_t[:, hl:hl + 1].to_broadcast([P, HD]))
                        nc.vector.tensor_add(kv_all[:, b + 1, :], kv_all[:, b + 1, :], c_ps[:])

                    for b in range(NB):
                        msl = slice(b * BLOCK, (b + 1) * BLOCK)
                        qk_ps = []
                        for no in range(2):
                            qk = ap_.tile([P, BLOCK], F32, tag="qk")
                            nc.tensor.matmul(qk[:], kT_s[:, b * BLOCK + no * P: b * BLOCK + (no + 1) * P],
                                             qT_s[:, msl], start=True, stop=True)
                            qk_ps.append(qk)
                        qkTm = asml.tile([P, 2, BLOCK], F32R, tag="qkm")
                        for no in range(2):
                            nc.vector.tensor_mul(qkTm[:, no, :], qk_ps[no][:], maskT_t[:, hl, no, :])
                        qdT = asml.tile([P, BLOCK], F32R, tag="qdT")
                        nc.vector.tensor_mul(qdT[:], qT_s[:, msl], qdec_t[:, hl, :])

                        o_ps = ap_.tile([P, BLOCK], F32, tag="o")
                        nc.tensor.matmul(o_ps[:], v_nat[:, 2 * b, :], qkTm[:, 0, :], start=True, stop=False)
                        nc.tensor.matmul(o_ps[:], v_nat[:, 2 * b + 1, :], qkTm[:, 1, :], start=False, stop=False)
                        nc.tensor.matmul(o_ps[:], kv_all[:, b, :], qdT[:], start=False, stop=True)
                        nc.any.tensor_copy(hiddenT_h[:, msl], o_ps[:])

                    # ssq partial + gated Y
                    sq_h = ah.tile([P, SEQ], F32R, tag="sq")
                    nc.vector.tensor_mul(sq_h[:], hiddenT_h[:], hiddenT_h[:])
                    for sc in range(SB):
                        sp = ap_.tile([P, P], F32, tag="C")
                        nc.tensor.matmul(sp[:], sq_h[:, sc * P:(sc + 1) * P], ones_t[:],
                                         start=True, stop=True)
                        if hl == 0:
                            nc.vector.tensor_copy(ssq_acc[:, sc:sc + 1], sp[:, 0:1])
                        else:
                            nc.vector.tensor_add(ssq_acc[:, sc:sc + 1], ssq_acc[:, sc:sc + 1], sp[:, 0:1])
                    if DBG:
                        nc.sync.dma_start(
                            dbg_hidT.ap().rearrange("(h p) s -> p h s", p=P)[:, hl, :], hiddenT_h[:])
                    y_h = ah.tile([P, SEQ], F32R, tag="y")
                    nc.vector.tensor_mul(y_h[:], hiddenT_h[:], gate_h[:])
                    for st in range(W):
                        nc.sync.dma_start(a2a_in[st, hl * P:(hl + 1) * P, :],
                                          y_h[:, st * SSH:(st + 1) * SSH])
                        if DBG:
                            nc.sync.dma_start(dbg_y.ap()[st, hl * P:(hl + 1) * P, :],
                                              y_h[:, st * SSH:(st + 1) * SSH])

            nc.sync.dma_start(ssq_in.rearrange("(a p) -> p a", p=P), ssq_acc[:])
            if DBG:
                nc.sync.dma_start(dbg_ssq.ap().rearrange("(a p) -> p a", p=P), ssq_acc[:])

            # ================= collectives ======================================
            if PHASES == "att":
                return _finish(nc)
            NOCOLL = bool(int(os.environ.get("KERNEL_NOCOLL", "0")))
            if NOCOLL:
                a2a_out = a2a_in
                ssq_out = ssq_in[:SSH]
            else:
                nc.gpsimd.collective_compute(
                    "AllToAll", ALU.bypass, replica_groups=[list(range(W))],
                    ins=[a2a_in.opt()], outs=[a2a_out.opt()])
                nc.gpsimd.collective_compute(
                    "ReduceScatter", ALU.add, replica_groups=[list(range(W))],
                    ins=[ssq_in.opt()], outs=[ssq_out.opt()])

            # rsqrt(var + eps)
            sq_raw = const.tile([P, SSH // P], F32)
            nc.sync.dma_start(sq_raw[:], ssq_out.rearrange("(i p) -> p i", p=P))
            t1 = const.tile([P, SSH // P], F32)
            nc.scalar.activation(t1[:], sq_raw[:], AF.Sqrt, bias=eps_t[:], scale=1.0 / INNER)
            nc.vector.reciprocal(rsq[:], t1[:])
            if DBG:
                nc.sync.dma_start(dbg_rsq.ap()[:], rsq[:])

            # ================= out projection (fp32r, seq-sharded) ==============
            with tc.tile_pool(name="oa", bufs=1) as oa, \
                 tc.tile_pool(name="ow", bufs=5) as ow, \
                 tc.tile_pool(name="oo", bufs=4) as oo, \
                 tc.tile_pool(name="op", bufs=4, space="PSUM") as opp:
                a2a_t = oa.tile([P, KO, SSH], F32R)
                a2a_v = a2a_out.rearrange("r (jo p) s -> p (r jo) s", p=P)
                for q in range(4):
                    nc.sync.dma_start(a2a_t[:, q * (KO // 4):(q + 1) * (KO // 4), :],
                                      a2a_v[:, q * (KO // 4):(q + 1) * (KO // 4), :])
                if DBG:
                    nc.sync.dma_start(dbg_a2a.ap().rearrange("r (jo p) s -> p (r jo) s", p=P), a2a_t[:])
                woutT_v = woutT.ap().rearrange("(ko p) o -> p ko o", p=P)
                out_v = out.ap().rearrange("(mo p) o -> p mo o", p=P)
                KC = KO // 4   # 8 k-subtiles per weight chunk
                for nt in range(H // 512):
                    osl = slice(nt * 512, (nt + 1) * 512)
                    w_ts = []
                    for q in range(4):
                        w_t = ow.tile([P, KC, 512], F32R, tag="w")
                        nc.sync.dma_start(w_t[:], woutT_v[:, q * KC:(q + 1) * KC, osl])
                        w_ts.append(w_t)
                    for mt in range(SSH // P):
                        ps = opp.tile([P, 512], F32, tag="po")
                        for kk in range(KO):
                            nc.tensor.matmul(ps[:], a2a_t[:, kk, mt * P:(mt + 1) * P],
                                             w_ts[kk // KC][:, kk % KC, :],
                                             start=(kk == 0), stop=(kk == KO - 1))
                        o_sb = oo.tile([P, 512], F32, tag="ot")
                        nc.vector.tensor_mul(o_sb[:], ps[:],
                                             rsq[:, mt:mt + 1].to_broadcast([P, 512]))
                        nc.sync.dma_start(out_v[:, mt, osl], o_sb[:])

    nc.compile()
    return nc


def _host_prep(inputs):
    x = np.asarray(inputs["x"], np.float32)
    w_qkv = np.asarray(inputs["w_qkv"], np.float32)
    w_gate = np.asarray(inputs["w_gate"], np.float32)
    w_out = np.asarray(inputs["w_out"], np.float32)
    norm_weight = np.asarray(inputs["norm_weight"], np.float32)
    kv_cache = np.asarray(inputs["kv_cache"], np.float32)
    slope = np.asarray(inputs["slope"], np.float32)

    bf = ml_dtypes.bfloat16
    xT_bf = np.ascontiguousarray(x.T).astype(bf)
    woutT = np.ascontiguousarray((w_out * norm_weight[None, :]).T).astype(np.float32)
    ident = np.eye(P, dtype=np.float32)
    ones = np.ones((P, P), np.float32)

    in_maps = []
    for c in range(W):
        sl = slope[c * HPC:(c + 1) * HPC]                     # [4]
        m0 = np.arange(BLOCK, dtype=np.float32)              # 0-based position in block
        # qdec[p, hl, m] = exp(-s*(m+1)) replicated over partitions
        qd = np.exp(-sl[:, None] * (m0[None, :] + 1.0))      # [4, 256]
        qdec_a = np.broadcast_to(qd[None], (P, HPC, BLOCK)).astype(np.float32).copy()
        # kdec[p, hl, no] = exp(-s*(BLOCK - (no*128+p+1)))
        n0 = (np.arange(2)[None, :] * P + np.arange(P)[:, None]).astype(np.float32)  # [128,2]
        kd = np.exp(-sl[None, None, :] * (BLOCK - (n0[:, :, None] + 1.0)))           # [128,2,4]
        kdec_a = np.ascontiguousarray(kd.transpose(0, 2, 1)).astype(np.float32)      # [128,4,2]
        # maskT[p, hl, no, m] = exp(-s*(m - n)) if m>=n else 0   (0-based n = no*128+p)
        nfull = n0[:, :, None]                                # [128,2,1]
        diff = m0[None, None, :] - nfull                      # [128,2,256]
        dif4 = diff[..., None]                                # [128,2,256,1]
        mask = np.where(dif4 >= 0,
                        np.exp(-sl[None, None, None, :] * np.maximum(dif4, 0.0)),
                        0.0)                                  # [128,2,256,4]
        maskT_a = np.ascontiguousarray(mask.transpose(0, 3, 1, 2)).astype(np.float32)        # [128,4,2,256]
        blkdec_a = np.broadcast_to(np.exp(-sl * BLOCK)[None], (P, HPC)).astype(np.float32).copy()

        in_maps.append({
            "xT": xT_bf,
            "wqkvT": np.ascontiguousarray(w_qkv[MPC * c:MPC * (c + 1)].T).astype(bf),
            "wgateT": np.ascontiguousarray(w_gate[JPC * c:JPC * (c + 1)].T).astype(bf),
            "woutT": woutT,
            "qdec": qdec_a,
            "kdec": kdec_a,
            "maskT": maskT_a,
            "blkdec": blkdec_a,
            "ident_r": ident,
            "ones_r": ones,
            "eps_b": np.full((P, 1), EPS, np.float32),
            "kv0": np.ascontiguousarray(kv_cache[HPC * c:HPC * (c + 1)]),
        })
    return in_maps


_CACHE = {}


def _get_program():
    if "nc" not in _CACHE:
        _CACHE["nc"] = _build_program()
    return _CACHE["nc"]


def kernel(**inputs):
    nc = _get_program()
    in_maps = _host_prep(inputs)
    trace = bool(int(os.environ.get("KERNEL_TRACE", "0")))
    res = run_bass_kernel_spmd(nc, in_maps, core_ids=list(range(W)), trace=trace)
    _CACHE["last_results"] = res
    out = np.concatenate([res.results[c]["out"] for c in range(W)], axis=0)
    return out.astype(np.float32)


# revision 21
# speedup vs baseline: 1.0970x; 1.0482x over previous
"""MiniMaxText01 linear attention (lightning attention) prefill layer on 8 TRN2 NeuronCores.

Strategy: tensor-parallel over heads (4 heads/core) for qkv+gate+attention;
AllToAll to switch to sequence-parallel for the output projection;
ReduceScatter for the RMSNorm variance. See spec sharding_hint.
"""
import os
import sys
import math

sys.path.insert(0, "/opt/trn_rl_repo")

import numpy as np
import ml_dtypes

import concourse.bass as bass
import concourse.mybir as mybir
import concourse.tile as tile
from concourse import bacc
from concourse.bass_utils import run_bass_kernel_spmd

# problem constants (hardcoded per contract)
H = 4096
INNER = 4096
NH = 32
HD = 128
SEQ = 4096
BLOCK = 256
EPS = 1e-5
P = 128
W = 8                    # cores
HPC = NH // W            # heads per core = 4
MPC = 3 * HD * HPC       # qkv rows per core = 1536
JPC = HD * HPC           # inner cols per core = 512
SSH = SEQ // W           # seq shard = 512
KO = H // P              # 32 k-subtiles
NB = SEQ // BLOCK        # 16 blocks
SB = SEQ // P            # 32 sub-blocks of 128

F32 = mybir.dt.float32
F32R = mybir.dt.float32r
BF16 = mybir.dt.bfloat16
AF = mybir.ActivationFunctionType
ALU = mybir.AluOpType


def _finish(nc):
    return nc


def _build_program():
    nc = bacc.Bacc("TRN2", target_bir_lowering=False, debug=False, num_devices=W)

    # ---- I/O ----
    xT = nc.dram_tensor("xT", [H, SEQ], BF16, kind="ExternalInput")
    wqkvT = nc.dram_tensor("wqkvT", [H, MPC], BF16, kind="ExternalInput")
    wgateT = nc.dram_tensor("wgateT", [H, JPC], BF16, kind="ExternalInput")
    woutT = nc.dram_tensor("woutT", [INNER, H], BF16, kind="ExternalInput")
    qdec = nc.dram_tensor("qdec", [P, HPC, BLOCK], F32, kind="ExternalInput")
    kdec = nc.dram_tensor("kdec", [P, HPC, 2], F32, kind="ExternalInput")
    maskT = nc.dram_tensor("maskT", [P, HPC, 2, BLOCK], F32, kind="ExternalInput")
    blkdec = nc.dram_tensor("blkdec", [P, HPC], F32, kind="ExternalInput")
    ident_r = nc.dram_tensor("ident_r", [P, P], F32R, kind="ExternalInput")
    ones_r = nc.dram_tensor("ones_r", [P, P], BF16, kind="ExternalInput")
    eps_b = nc.dram_tensor("eps_b", [P, 1], F32, kind="ExternalInput")
    kv0 = nc.dram_tensor("kv0", [HPC, HD, HD], F32, kind="ExternalInput")
    out = nc.dram_tensor("out", [SSH, H], F32, kind="ExternalOutput")
    DBG = bool(int(os.environ.get("KERNEL_DEBUG", "0")))
    if DBG:
        dbg_qkvT = nc.dram_tensor("dbg_qkvT", [MPC, SEQ], F32R, kind="ExternalOutput")
        dbg_gateT = nc.dram_tensor("dbg_gateT", [JPC, SEQ], F32, kind="ExternalOutput")
        dbg_hidT = nc.dram_tensor("dbg_hidT", [JPC, SEQ], F32, kind="ExternalOutput")
        dbg_y = nc.dram_tensor("dbg_y", [W, JPC, SSH], BF16, kind="ExternalOutput")
        dbg_a2a = nc.dram_tensor("dbg_a2a", [W, JPC, SSH], BF16, kind="ExternalOutput")
        dbg_ssq = nc.dram_tensor("dbg_ssq", [SEQ], F32, kind="ExternalOutput")
        dbg_rsq = nc.dram_tensor("dbg_rsq", [P, SSH // P], F32, kind="ExternalOutput")

    with tile.TileContext(nc) as tc:
        with tc.tile_pool(name="dram", bufs=1, space="DRAM") as dram, \
             tc.tile_pool(name="const", bufs=1) as const:
            # ---- DRAM temporaries ----
            qkvT_d0 = dram.tile([MPC // 2, SEQ], F32R)           # silu(qkv) heads 0-1, transposed
            qkvT_d1 = dram.tile([MPC // 2, SEQ], F32R)           # heads 2-3
            gateT_d = dram.tile([JPC, SEQ], F32)                 # sigmoid gate, transposed
            a2a_in = dram.tile([W, JPC, SSH], BF16)              # Y shards (j-major per shard)
            a2a_out = dram.tile([W, JPC, SSH], BF16)
            ssq_in = dram.tile([SEQ], F32)
            ssq_out = dram.tile([SSH], F32)

            # ---- constants in SBUF ----
            qdec_t = const.tile([P, HPC, BLOCK], F32)
            nc.sync.dma_start(qdec_t[:], qdec.ap()[:])
            kdec_t = const.tile([P, HPC, 2], F32)
            nc.sync.dma_start(kdec_t[:], kdec.ap()[:])
            maskT_t = const.tile([P, HPC, 2, BLOCK], F32)
            nc.sync.dma_start(maskT_t[:], maskT.ap()[:])
            blkdec_t = const.tile([P, HPC], F32)
            nc.sync.dma_start(blkdec_t[:], blkdec.ap()[:])
            ident_t = const.tile([P, P], F32R)
            nc.sync.dma_start(ident_t[:], ident_r.ap()[:])
            ones_t = const.tile([P, P], BF16)
            nc.sync.dma_start(ones_t[:], ones_r.ap()[:])
            eps_t = const.tile([P, 1], F32)
            nc.sync.dma_start(eps_t[:], eps_b.ap()[:])
            ssq_acc = const.tile([P, SB], F32)
            rsq = const.tile([P, SSH // P], F32)

            xT_v = xT.ap().rearrange("(ko p) s -> p ko s", p=P)
            wqkvT_v = wqkvT.ap().rearrange("(ko p) m -> p ko m", p=P)
            wgateT_v = wgateT.ap().rearrange("(ko p) m -> p ko m", p=P)
            qkvT_v0 = qkvT_d0.rearrange("(mo p) s -> p mo s", p=P)
            qkvT_v1 = qkvT_d1.rearrange("(mo p) s -> p mo s", p=P)
            gateT_v = gateT_d.rearrange("(go p) s -> p go s", p=P)

            # ================= phase AB: qkv + gate projections (bf16) ==========
            MG = MPC // 2   # 768 cols of wqkvT per group
            KC = 8          # k-subtiles per chunk (4 chunks cover K)
            NKC = KO // KC
            with tc.tile_pool(name="abw", bufs=NKC + 1) as wp, \
                 tc.tile_pool(name="abg", bufs=NKC) as gp, \
                 tc.tile_pool(name="abx", bufs=NKC + 2) as xp, \
                 tc.tile_pool(name="abo", bufs=4) as op, \
                 tc.tile_pool(name="abp", bufs=4, space="PSUM") as pp:
                wg_c = []
                for kc in range(NKC):
                    wg_t = gp.tile([P, KC, JPC], BF16, tag="wg", name=f"wg{kc}")
                    nc.sync.dma_start(wg_t[:], wgateT_v[:, kc * KC:(kc + 1) * KC, :])
                    wg_c.append(wg_t)
                for grp in range(2):
                    wq_c = []
                    for kc in range(NKC):
                        wq_t = wp.tile([P, KC, MG], BF16, tag="wq", name=f"wq{grp}_{kc}")
                        nc.sync.dma_start(wq_t[:], wqkvT_v[:, kc * KC:(kc + 1) * KC,
                                                           grp * MG:(grp + 1) * MG])
                        wq_c.append(wq_t)
                    for n in range(SEQ // 512):
                        x_c = []
                        for kc in range(NKC):
                            x_t = xp.tile([P, KC, 512], BF16, tag="x", name=f"x{grp}_{n}_{kc}")
                            nc.sync.dma_start(x_t[:], xT_v[:, kc * KC:(kc + 1) * KC,
                                                           n * 512:(n + 1) * 512])
                            x_c.append(x_t)
                        for mm in range(MG // P):
                            ps = pp.tile([P, 512], F32, tag="ps")
                            for k in range(KO):
                                nc.tensor.matmul(ps[:], wq_c[k // KC][:, k % KC, mm * P:(mm + 1) * P],
                                                 x_c[k // KC][:, k % KC, :],
                                                 start=(k == 0), stop=(k == KO - 1))
                            o_t = op.tile([P, 512], F32R, tag="o")
                            nc.scalar.activation(o_t[:], ps[:], AF.Silu)
                            qv = qkvT_v0 if grp == 0 else qkvT_v1
                            nc.sync.dma_start(qv[:, mm, n * 512:(n + 1) * 512], o_t[:])
                            if DBG:
                                m_global = grp * (MG // P) + mm
                                nc.sync.dma_start(
                                    dbg_qkvT.ap().rearrange("(mo p) s -> p mo s", p=P)[:, m_global, n * 512:(n + 1) * 512], o_t[:])
                        if grp == 0:
                            for gg in range(JPC // P):
                                ps = pp.tile([P, 512], F32, tag="ps")
                                for k in range(KO):
                                    nc.tensor.matmul(ps[:], wg_c[k // KC][:, k % KC, gg * P:(gg + 1) * P],
                                                     x_c[k // KC][:, k % KC, :],
                                                     start=(k == 0), stop=(k == KO - 1))
                                g_t = op.tile([P, 512], F32, tag="g")
                                nc.scalar.activation(g_t[:], ps[:], AF.Sigmoid)
                                nc.sync.dma_start(gateT_v[:, gg, n * 512:(n + 1) * 512], g_t[:])
                                if DBG:
                                    nc.sync.dma_start(
                                        dbg_gateT.ap().rearrange("(go p) s -> p go s", p=P)[:, gg, n * 512:(n + 1) * 512], g_t[:])

            PHASES = os.environ.get("KERNEL_PHASES", "full")
            # ================= attention (fp32r) per head =======================
            if PHASES == "ab":
                return _finish(nc)
            with tc.tile_pool(name="atth", bufs=1) as ah, \
                 tc.tile_pool(name="atts", bufs=3) as asml, \
                 tc.tile_pool(name="attp", bufs=2, space="PSUM") as ap_:
                for hl in range(HPC):
                    qv = qkvT_v0 if hl < 2 else qkvT_v1
                    mo0 = 3 * (hl % 2)
                    qT_s = ah.tile([P, SEQ], F32R, tag="qT")
                    nc.sync.dma_start(qT_s[:], qv[:, mo0 + 0, :])
                    kT_s = ah.tile([P, SEQ], F32R, tag="kT")
                    nc.sync.dma_start(kT_s[:], qv[:, mo0 + 1, :])
                    vT_s = ah.tile([P, SEQ], F32R, tag="vT")
                    nc.sync.dma_start(vT_s[:], qv[:, mo0 + 2, :])
                    gate_h = ah.tile([P, SEQ], F32, tag="gate")
                    nc.sync.dma_start(gate_h[:], gateT_v[:, hl, :])
                    kv_state = ah.tile([P, HD], F32, tag="kv")
                    nc.sync.dma_start(kv_state[:], kv0.ap()[hl])

                    # k/v natural layouts via PE transpose; kdec folded into k copy
                    kd_nat = ah.tile([P, SB, HD], F32R, tag="kd")
                    v_nat = ah.tile([P, SB, HD], F32R, tag="vn")
                    for sb in range(SB):
                        pst = ap_.tile([P, P], F32R, tag="tr")
                        nc.tensor.transpose(pst[:], kT_s[:, sb * P:(sb + 1) * P], ident_t[:])
                        nc.scalar.activation(kd_nat[:, sb, :], pst[:], AF.Copy,
                                             scale=kdec_t[:, hl, sb % 2:sb % 2 + 1])
                        pst2 = ap_.tile([P, P], F32R, tag="tr")
                        nc.tensor.transpose(pst2[:], vT_s[:, sb * P:(sb + 1) * P], ident_t[:])
                        nc.any.tensor_copy(v_nat[:, sb, :], pst2[:])

                    hiddenT_h = ah.tile([P, SEQ], F32, tag="hid")

                    # kv prefix scan: kv_all[:, b] = state BEFORE block b. C matmuls
                    # are independent; only the cheap DVE scan is serial, so the
                    # o-matmuls below never wait on the recurrence.
                    kv_all = ah.tile([P, NB, HD], F32R, tag="kva")
                    nc.scalar.activation(kv_all[:, 0, :], kv_state[:], AF.Copy)
                    for b in range(NB - 1):
                        c_ps = ap_.tile([P, HD], F32, tag="C")
                        nc.tensor.matmul(c_ps[:], kd_nat[:, 2 * b, :], v_nat[:, 2 * b, :], start=True, stop=False)
                        nc.tensor.matmul(c_ps[:], kd_nat[:, 2 * b + 1, :], v_nat[:, 2 * b + 1, :], start=False, stop=True)
                        nc.vector.tensor_mul(kv_all[:, b + 1, :], kv_all[:, b, :],
                                             blkdec_t[:, hl:hl + 1].to_broadcast([P, HD]))
                        nc.vector.tensor_add(kv_all[:, b + 1, :], kv_all[:, b + 1, :], c_ps[:])

                    for b in range(NB):
                        msl = slice(b * BLOCK, (b + 1) * BLOCK)
                        qk_ps = []
                        for no in range(2):
                            qk = ap_.tile([P, BLOCK], F32, tag="qk")
                            nc.tensor.matmul(qk[:], kT_s[:, b * BLOCK + no * P: b * BLOCK + (no + 1) * P],
                                             qT_s[:, msl], start=True, stop=True)
                            qk_ps.append(qk)
                        qkTm = asml.tile([P, 2, BLOCK], F32R, tag="qkm")
                        for no in range(2):
                            nc.vector.tensor_mul(qkTm[:, no, :], qk_ps[no][:], maskT_t[:, hl, no, :])
                        qdT = asml.tile([P, BLOCK], F32R, tag="qdT")
                        nc.vector.tensor_mul(qdT[:], qT_s[:, msl], qdec_t[:, hl, :])

                        o_ps = ap_.tile([P, BLOCK], F32, tag="o")
                        nc.tensor.matmul(o_ps[:], v_nat[:, 2 * b, :], qkTm[:, 0, :], start=True, stop=False)
                        nc.tensor.matmul(o_ps[:], v_nat[:, 2 * b + 1, :], qkTm[:, 1, :], start=False, stop=False)
                        nc.tensor.matmul(o_ps[:], kv_all[:, b, :], qdT[:], start=False, stop=True)
                        nc.any.tensor_copy(hiddenT_h[:, msl], o_ps[:])

                    # ssq partial + gated Y
                    sq_h = ah.tile([P, SEQ], BF16, tag="sq")
                    nc.vector.tensor_mul(sq_h[:], hiddenT_h[:], hiddenT_h[:])
                    for sc in range(SB):
                        sp = ap_.tile([P, P], F32, tag="C")
                        nc.tensor.matmul(sp[:], sq_h[:, sc * P:(sc + 1) * P], ones_t[:],
                                         start=True, stop=True)
                        if hl == 0:
                            nc.vector.tensor_copy(ssq_acc[:, sc:sc + 1], sp[:, 0:1])
                        else:
                            nc.vector.tensor_add(ssq_acc[:, sc:sc + 1], ssq_acc[:, sc:sc + 1], sp[:, 0:1])
                    if DBG:
                        nc.sync.dma_start(
                            dbg_hidT.ap().rearrange("(h p) s -> p h s", p=P)[:, hl, :], hiddenT_h[:])
                    y_h = ah.tile([P, SEQ], BF16, tag="y")
                    nc.vector.tensor_mul(y_h[:], hiddenT_h[:], gate_h[:])
                    for st in range(W):
                        nc.sync.dma_start(a2a_in[st, hl * P:(hl + 1) * P, :],
                                          y_h[:, st * SSH:(st + 1) * SSH])
                        if DBG:
                            nc.sync.dma_start(dbg_y.ap()[st, hl * P:(hl + 1) * P, :],
                                              y_h[:, st * SSH:(st + 1) * SSH])

            nc.sync.dma_start(ssq_in.rearrange("(a p) -> p a", p=P), ssq_acc[:])
            if DBG:
                nc.sync.dma_start(dbg_ssq.ap().rearrange("(a p) -> p a", p=P), ssq_acc[:])

            # ================= collectives ======================================
            if PHASES == "att":
                return _finish(nc)
            NOCOLL = bool(int(os.environ.get("KERNEL_NOCOLL", "0")))
            if NOCOLL:
                a2a_out = a2a_in
                ssq_out = ssq_in[:SSH]
            else:
                nc.gpsimd.collective_compute(
                    "AllToAll", ALU.bypass, replica_groups=[list(range(W))],
                    ins=[a2a_in.opt()], outs=[a2a_out.opt()])
                nc.gpsimd.collective_compute(
                    "ReduceScatter", ALU.add, replica_groups=[list(range(W))],
                    ins=[ssq_in.opt()], outs=[ssq_out.opt()])

            # rsqrt(var + eps)
            sq_raw = const.tile([P, SSH // P], F32)
            nc.sync.dma_start(sq_raw[:], ssq_out.rearrange("(i p) -> p i", p=P))
            t1 = const.tile([P, SSH // P], F32)
            nc.scalar.activation(t1[:], sq_raw[:], AF.Sqrt, bias=eps_t[:], scale=1.0 / INNER)
            nc.vector.reciprocal(rsq[:], t1[:])
            if DBG:
                nc.sync.dma_start(dbg_rsq.ap()[:], rsq[:])

            # ================= out projection (fp32r, seq-sharded) ==============
            with tc.tile_pool(name="oa", bufs=1) as oa, \
                 tc.tile_pool(name="ow", bufs=5) as ow, \
                 tc.tile_pool(name="oo", bufs=4) as oo, \
                 tc.tile_pool(name="op", bufs=4, space="PSUM") as opp:
                a2a_t = oa.tile([P, KO, SSH], BF16)
                a2a_v = a2a_out.rearrange("r (jo p) s -> p (r jo) s", p=P)
                for q in range(4):
                    nc.sync.dma_start(a2a_t[:, q * (KO // 4):(q + 1) * (KO // 4), :],
                                      a2a_v[:, q * (KO // 4):(q + 1) * (KO // 4), :])
                if DBG:
                    nc.sync.dma_start(dbg_a2a.ap().rearrange("r (jo p) s -> p (r jo) s", p=P), a2a_t[:])
                woutT_v = woutT.ap().rearrange("(ko p) o -> p ko o", p=P)
                out_v = out.ap().rearrange("(mo p) o -> p mo o", p=P)
                KC = KO // 4   # 8 k-subtiles per weight chunk
                for nt in range(H // 512):
                    osl = slice(nt * 512, (nt + 1) * 512)
                    w_ts = []
                    for q in range(4):
                        w_t = ow.tile([P, KC, 512], BF16, tag="w")
                        nc.sync.dma_start(w_t[:], woutT_v[:, q * KC:(q + 1) * KC, osl])
                        w_ts.append(w_t)
                    for mt in range(SSH // P):
                        ps = opp.tile([P, 512], F32, tag="po")
                        for kk in range(KO):
                            nc.tensor.matmul(ps[:], a2a_t[:, kk, mt * P:(mt + 1) * P],
                                             w_ts[kk // KC][:, kk % KC, :],
                                             start=(kk == 0), stop=(kk == KO - 1))
                        o_sb = oo.tile([P, 512], F32, tag="ot")
                        nc.vector.tensor_mul(o_sb[:], ps[:],
                                             rsq[:, mt:mt + 1].to_broadcast([P, 512]))
                        nc.sync.dma_start(out_v[:, mt, osl], o_sb[:])

    nc.compile()
    return nc


def _host_prep(inputs):
    x = np.asarray(inputs["x"], np.float32)
    w_qkv = np.asarray(inputs["w_qkv"], np.float32)
    w_gate = np.asarray(inputs["w_gate"], np.float32)
    w_out = np.asarray(inputs["w_out"], np.float32)
    norm_weight = np.asarray(inputs["norm_weight"], np.float32)
    kv_cache = np.asarray(inputs["kv_cache"], np.float32)
    slope = np.asarray(inputs["slope"], np.float32)

    bf = ml_dtypes.bfloat16
    xT_bf = np.ascontiguousarray(x.T).astype(bf)
    woutT = np.ascontiguousarray((w_out * norm_weight[None, :]).T).astype(bf)
    ident = np.eye(P, dtype=np.float32)
    ones = np.ones((P, P), ml_dtypes.bfloat16)

    in_maps = []
    for c in range(W):
        sl = slope[c * HPC:(c + 1) * HPC]                     # [4]
        m0 = np.arange(BLOCK, dtype=np.float32)              # 0-based position in block
        # qdec[p, hl, m] = exp(-s*(m+1)) replicated over partitions
        qd = np.exp(-sl[:, None] * (m0[None, :] + 1.0))      # [4, 256]
        qdec_a = np.broadcast_to(qd[None], (P, HPC, BLOCK)).astype(np.float32).copy()
        # kdec[p, hl, no] = exp(-s*(BLOCK - (no*128+p+1)))
        n0 = (np.arange(2)[None, :] * P + np.arange(P)[:, None]).astype(np.float32)  # [128,2]
        kd = np.exp(-sl[None, None, :] * (BLOCK - (n0[:, :, None] + 1.0)))           # [128,2,4]
        kdec_a = np.ascontiguousarray(kd.transpose(0, 2, 1)).astype(np.float32)      # [128,4,2]
        # maskT[p, hl, no, m] = exp(-s*(m - n)) if m>=n else 0   (0-based n = no*128+p)
        nfull = n0[:, :, None]                                # [128,2,1]
        diff = m0[None, None, :] - nfull                      # [128,2,256]
        dif4 = diff[..., None]                                # [128,2,256,1]
        mask = np.where(dif4 >= 0,
                        np.exp(-sl[None, None, None, :] * np.maximum(dif4, 0.0)),
                        0.0)                                  # [128,2,256,4]
        maskT_a = np.ascontiguousarray(mask.transpose(0, 3, 1, 2)).astype(np.float32)        # [128,4,2,256]
        blkdec_a = np.broadcast_to(np.exp(-sl * BLOCK)[None], (P, HPC)).astype(np.float32).copy()

        in_maps.append({
            "xT": xT_bf,
            "wqkvT": np.ascontiguousarray(w_qkv[MPC * c:MPC * (c + 1)].T).astype(bf),
            "wgateT": np.ascontiguousarray(w_gate[JPC * c:JPC * (c + 1)].T).astype(bf),
            "woutT": woutT,
            "qdec": qdec_a,
            "kdec": kdec_a,
            "maskT": maskT_a,
            "blkdec": blkdec_a,
            "ident_r": ident,
            "ones_r": ones,
            "eps_b": np.full((P, 1), EPS, np.float32),
            "kv0": np.ascontiguousarray(kv_cache[HPC * c:HPC * (c + 1)]),
        })
    return in_maps


_CACHE = {}


def _get_program():
    if "nc" not in _CACHE:
        _CACHE["nc"] = _build_program()
    return _CACHE["nc"]


def kernel(**inputs):
    nc = _get_program()
    in_maps = _host_prep(inputs)
    trace = bool(int(os.environ.get("KERNEL_TRACE", "0")))
    res = run_bass_kernel_spmd(nc, in_maps, core_ids=list(range(W)), trace=trace)
    _CACHE["last_results"] = res
    out = np.concatenate([res.results[c]["out"] for c in range(W)], axis=0)
    return out.astype(np.float32)


# revision 22
# speedup vs baseline: 1.1000x; 1.0027x over previous
"""MiniMaxText01 linear attention (lightning attention) prefill layer on 8 TRN2 NeuronCores.

Strategy: tensor-parallel over heads (4 heads/core) for qkv+gate+attention;
AllToAll to switch to sequence-parallel for the output projection;
ReduceScatter for the RMSNorm variance. See spec sharding_hint.
"""
import os
import sys
import math

sys.path.insert(0, "/opt/trn_rl_repo")

import numpy as np
import ml_dtypes

import concourse.bass as bass
import concourse.mybir as mybir
import concourse.tile as tile
from concourse import bacc
from concourse.bass_utils import run_bass_kernel_spmd

# problem constants (hardcoded per contract)
H = 4096
INNER = 4096
NH = 32
HD = 128
SEQ = 4096
BLOCK = 256
EPS = 1e-5
P = 128
W = 8                    # cores
HPC = NH // W            # heads per core = 4
MPC = 3 * HD * HPC       # qkv rows per core = 1536
JPC = HD * HPC           # inner cols per core = 512
SSH = SEQ // W           # seq shard = 512
KO = H // P              # 32 k-subtiles
NB = SEQ // BLOCK        # 16 blocks
SB = SEQ // P            # 32 sub-blocks of 128

F32 = mybir.dt.float32
F32R = mybir.dt.float32r
BF16 = mybir.dt.bfloat16
AF = mybir.ActivationFunctionType
ALU = mybir.AluOpType


def _finish(nc):
    return nc


def _build_program():
    nc = bacc.Bacc("TRN2", target_bir_lowering=False, debug=False, num_devices=W)

    # ---- I/O ----
    xT = nc.dram_tensor("xT", [H, SEQ], BF16, kind="ExternalInput")
    wqkvT = nc.dram_tensor("wqkvT", [H, MPC], BF16, kind="ExternalInput")
    wgateT = nc.dram_tensor("wgateT", [H, JPC], BF16, kind="ExternalInput")
    woutT = nc.dram_tensor("woutT", [INNER, H], BF16, kind="ExternalInput")
    qdec = nc.dram_tensor("qdec", [P, HPC, BLOCK], F32, kind="ExternalInput")
    kdec = nc.dram_tensor("kdec", [P, HPC, 2], F32, kind="ExternalInput")
    maskT = nc.dram_tensor("maskT", [P, HPC, 2, BLOCK], F32, kind="ExternalInput")
    blkdec = nc.dram_tensor("blkdec", [P, HPC], F32, kind="ExternalInput")
    ident_r = nc.dram_tensor("ident_r", [P, P], F32R, kind="ExternalInput")
    ones_r = nc.dram_tensor("ones_r", [P, P], BF16, kind="ExternalInput")
    eps_b = nc.dram_tensor("eps_b", [P, 1], F32, kind="ExternalInput")
    kv0 = nc.dram_tensor("kv0", [HPC, HD, HD], F32, kind="ExternalInput")
    out = nc.dram_tensor("out", [SSH, H], F32, kind="ExternalOutput")
    DBG = bool(int(os.environ.get("KERNEL_DEBUG", "0")))
    if DBG:
        dbg_qkvT = nc.dram_tensor("dbg_qkvT", [MPC, SEQ], F32R, kind="ExternalOutput")
        dbg_gateT = nc.dram_tensor("dbg_gateT", [JPC, SEQ], F32, kind="ExternalOutput")
        dbg_hidT = nc.dram_tensor("dbg_hidT", [JPC, SEQ], F32, kind="ExternalOutput")
        dbg_y = nc.dram_tensor("dbg_y", [W, JPC, SSH], BF16, kind="ExternalOutput")
        dbg_a2a = nc.dram_tensor("dbg_a2a", [W, JPC, SSH], BF16, kind="ExternalOutput")
        dbg_ssq = nc.dram_tensor("dbg_ssq", [SEQ], F32, kind="ExternalOutput")
        dbg_rsq = nc.dram_tensor("dbg_rsq", [P, SSH // P], F32, kind="ExternalOutput")

    with tile.TileContext(nc) as tc:
        with tc.tile_pool(name="dram", bufs=1, space="DRAM") as dram, \
             tc.tile_pool(name="const", bufs=1) as const:
            # ---- DRAM temporaries ----
            qkvT_d0 = dram.tile([MPC // 2, SEQ], F32R)           # silu(qkv) heads 0-1, transposed
            qkvT_d1 = dram.tile([MPC // 2, SEQ], F32R)           # heads 2-3
            gateT_d = dram.tile([JPC, SEQ], F32)                 # sigmoid gate, transposed
            a2a_in = dram.tile([W, JPC, SSH], BF16)              # Y shards (j-major per shard)
            a2a_out = dram.tile([W, JPC, SSH], BF16)
            ssq_in = dram.tile([SEQ], F32)
            ssq_out = dram.tile([SSH], F32)

            # ---- constants in SBUF ----
            qdec_t = const.tile([P, HPC, BLOCK], F32)
            nc.sync.dma_start(qdec_t[:], qdec.ap()[:])
            kdec_t = const.tile([P, HPC, 2], F32)
            nc.sync.dma_start(kdec_t[:], kdec.ap()[:])
            maskT_t = const.tile([P, HPC, 2, BLOCK], F32)
            nc.sync.dma_start(maskT_t[:], maskT.ap()[:])
            blkdec_t = const.tile([P, HPC], F32)
            nc.sync.dma_start(blkdec_t[:], blkdec.ap()[:])
            ident_t = const.tile([P, P], F32R)
            nc.sync.dma_start(ident_t[:], ident_r.ap()[:])
            ones_t = const.tile([P, P], BF16)
            nc.sync.dma_start(ones_t[:], ones_r.ap()[:])
            eps_t = const.tile([P, 1], F32)
            nc.sync.dma_start(eps_t[:], eps_b.ap()[:])
            ssq_acc = const.tile([P, SB], F32)
            rsq = const.tile([P, SSH // P], F32)

            xT_v = xT.ap().rearrange("(ko p) s -> p ko s", p=P)
            wqkvT_v = wqkvT.ap().rearrange("(ko p) m -> p ko m", p=P)
            wgateT_v = wgateT.ap().rearrange("(ko p) m -> p ko m", p=P)
            qkvT_v0 = qkvT_d0.rearrange("(mo p) s -> p mo s", p=P)
            qkvT_v1 = qkvT_d1.rearrange("(mo p) s -> p mo s", p=P)
            gateT_v = gateT_d.rearrange("(go p) s -> p go s", p=P)

            # ================= phase AB: qkv + gate projections (bf16) ==========
            MG = MPC // 2   # 768 cols of wqkvT per group
            KC = 8          # k-subtiles per chunk (4 chunks cover K)
            NKC = KO // KC
            with tc.tile_pool(name="abw", bufs=NKC + 1) as wp, \
                 tc.tile_pool(name="abg", bufs=NKC) as gp, \
                 tc.tile_pool(name="abx", bufs=NKC + 2) as xp, \
                 tc.tile_pool(name="abo", bufs=4) as op, \
                 tc.tile_pool(name="abp", bufs=4, space="PSUM") as pp:
                wg_c = []
                for kc in range(NKC):
                    wg_t = gp.tile([P, KC, JPC], BF16, tag="wg", name=f"wg{kc}")
                    nc.sync.dma_start(wg_t[:], wgateT_v[:, kc * KC:(kc + 1) * KC, :])
                    wg_c.append(wg_t)
                for grp in range(2):
                    wq_c = []
                    for kc in range(NKC):
                        wq_t = wp.tile([P, KC, MG], BF16, tag="wq", name=f"wq{grp}_{kc}")
                        nc.sync.dma_start(wq_t[:], wqkvT_v[:, kc * KC:(kc + 1) * KC,
                                                           grp * MG:(grp + 1) * MG])
                        wq_c.append(wq_t)
                    for n in range(SEQ // 512):
                        x_c = []
                        for kc in range(NKC):
                            x_t = xp.tile([P, KC, 512], BF16, tag="x", name=f"x{grp}_{n}_{kc}")
                            nc.sync.dma_start(x_t[:], xT_v[:, kc * KC:(kc + 1) * KC,
                                                           n * 512:(n + 1) * 512])
                            x_c.append(x_t)
                        for mm in range(MG // P):
                            ps = pp.tile([P, 512], F32, tag="ps")
                            for k in range(KO):
                                nc.tensor.matmul(ps[:], wq_c[k // KC][:, k % KC, mm * P:(mm + 1) * P],
                                                 x_c[k // KC][:, k % KC, :],
                                                 start=(k == 0), stop=(k == KO - 1))
                            o_t = op.tile([P, 512], F32R, tag="o")
                            nc.scalar.activation(o_t[:], ps[:], AF.Silu)
                            qv = qkvT_v0 if grp == 0 else qkvT_v1
                            nc.sync.dma_start(qv[:, mm, n * 512:(n + 1) * 512], o_t[:])
                            if DBG:
                                m_global = grp * (MG // P) + mm
                                nc.sync.dma_start(
                                    dbg_qkvT.ap().rearrange("(mo p) s -> p mo s", p=P)[:, m_global, n * 512:(n + 1) * 512], o_t[:])
                        if grp == 0:
                            for gg in range(JPC // P):
                                ps = pp.tile([P, 512], F32, tag="ps")
                                for k in range(KO):
                                    nc.tensor.matmul(ps[:], wg_c[k // KC][:, k % KC, gg * P:(gg + 1) * P],
                                                     x_c[k // KC][:, k % KC, :],
                                                     start=(k == 0), stop=(k == KO - 1))
                                g_t = op.tile([P, 512], F32, tag="g")
                                nc.scalar.activation(g_t[:], ps[:], AF.Sigmoid)
                                nc.sync.dma_start(gateT_v[:, gg, n * 512:(n + 1) * 512], g_t[:])
                                if DBG:
                                    nc.sync.dma_start(
                                        dbg_gateT.ap().rearrange("(go p) s -> p go s", p=P)[:, gg, n * 512:(n + 1) * 512], g_t[:])

            PHASES = os.environ.get("KERNEL_PHASES", "full")
            # ================= attention (fp32r) per head =======================
            if PHASES == "ab":
                return _finish(nc)
            with tc.tile_pool(name="atth", bufs=1) as ah, \
                 tc.tile_pool(name="atts", bufs=3) as asml, \
                 tc.tile_pool(name="attp", bufs=2, space="PSUM") as ap_:
                for hl in range(HPC):
                    qv = qkvT_v0 if hl < 2 else qkvT_v1
                    mo0 = 3 * (hl % 2)
                    gate_h = ah.tile([P, SEQ], F32, tag="gate")
                    nc.sync.dma_start(gate_h[:], gateT_v[:, hl, :])
                    kv_state = ah.tile([P, HD], F32, tag="kv")
                    nc.sync.dma_start(kv_state[:], kv0.ap()[hl])
                    hiddenT_h = ah.tile([P, SEQ], F32, tag="hid")
                    # kv_all[:, b] = kv state BEFORE block b; updated one block ahead
                    # of its consumer so the o-matmuls never wait on the recurrence.
                    kv_all = ah.tile([P, NB, HD], F32R, tag="kva")
                    nc.scalar.activation(kv_all[:, 0, :], kv_state[:], AF.Copy)

                    for b in range(NB):
                        msl = slice(b * BLOCK, (b + 1) * BLOCK)
                        qkv_c = asml.tile([P, 3, BLOCK], F32R, tag="qkv_c", bufs=6)
                        nc.sync.dma_start(qkv_c[:], qv[:, mo0:mo0 + 3, msl])
                        kdvn = asml.tile([P, 2, 2, HD], F32R, tag="kdvn", bufs=6)
                        for no in range(2):
                            pst = ap_.tile([P, P], F32R, tag="tr")
                            nc.tensor.transpose(pst[:], qkv_c[:, 1, no * P:(no + 1) * P], ident_t[:])
                            nc.scalar.activation(kdvn[:, 0, no, :], pst[:], AF.Copy,
                                                 scale=kdec_t[:, hl, no:no + 1])
                            pst2 = ap_.tile([P, P], F32R, tag="tr")
                            nc.tensor.transpose(pst2[:], qkv_c[:, 2, no * P:(no + 1) * P], ident_t[:])
                            nc.any.tensor_copy(kdvn[:, 1, no, :], pst2[:])

                        if b < NB - 1:
                            c_ps = ap_.tile([P, HD], F32, tag="C")
                            nc.tensor.matmul(c_ps[:], kdvn[:, 0, 0, :], kdvn[:, 1, 0, :], start=True, stop=False)
                            nc.tensor.matmul(c_ps[:], kdvn[:, 0, 1, :], kdvn[:, 1, 1, :], start=False, stop=True)
                            nc.vector.tensor_mul(kv_all[:, b + 1, :], kv_all[:, b, :],
                                                 blkdec_t[:, hl:hl + 1].to_broadcast([P, HD]))
                            nc.vector.tensor_add(kv_all[:, b + 1, :], kv_all[:, b + 1, :], c_ps[:])

                        qk_ps = []
                        for no in range(2):
                            qk = ap_.tile([P, BLOCK], F32, tag="qk")
                            nc.tensor.matmul(qk[:], qkv_c[:, 1, no * P:(no + 1) * P],
                                             qkv_c[:, 0, :], start=True, stop=True)
                            qk_ps.append(qk)
                        qkTm = asml.tile([P, 2, BLOCK], F32R, tag="qkm")
                        for no in range(2):
                            nc.vector.tensor_mul(qkTm[:, no, :], qk_ps[no][:], maskT_t[:, hl, no, :])
                        qdT = asml.tile([P, BLOCK], F32R, tag="qdT")
                        nc.vector.tensor_mul(qdT[:], qkv_c[:, 0, :], qdec_t[:, hl, :])

                        o_ps = ap_.tile([P, BLOCK], F32, tag="o")
                        nc.tensor.matmul(o_ps[:], kdvn[:, 1, 0, :], qkTm[:, 0, :], start=True, stop=False)
                        nc.tensor.matmul(o_ps[:], kdvn[:, 1, 1, :], qkTm[:, 1, :], start=False, stop=False)
                        nc.tensor.matmul(o_ps[:], kv_all[:, b, :], qdT[:], start=False, stop=True)
                        nc.any.tensor_copy(hiddenT_h[:, msl], o_ps[:])

                    # ssq partial + gated Y
                    sq_h = ah.tile([P, SEQ], BF16, tag="sq")
                    nc.vector.tensor_mul(sq_h[:], hiddenT_h[:], hiddenT_h[:])
                    for sc in range(SB):
                        sp = ap_.tile([P, P], F32, tag="C")
                        nc.tensor.matmul(sp[:], sq_h[:, sc * P:(sc + 1) * P], ones_t[:],
                                         start=True, stop=True)
                        if hl == 0:
                            nc.vector.tensor_copy(ssq_acc[:, sc:sc + 1], sp[:, 0:1])
                        else:
                            nc.vector.tensor_add(ssq_acc[:, sc:sc + 1], ssq_acc[:, sc:sc + 1], sp[:, 0:1])
                    if DBG:
                        nc.sync.dma_start(
                            dbg_hidT.ap().rearrange("(h p) s -> p h s", p=P)[:, hl, :], hiddenT_h[:])
                    y_h = ah.tile([P, SEQ], BF16, tag="y")
                    nc.vector.tensor_mul(y_h[:], hiddenT_h[:], gate_h[:])
                    for st in range(W):
                        nc.sync.dma_start(a2a_in[st, hl * P:(hl + 1) * P, :],
                                          y_h[:, st * SSH:(st + 1) * SSH])
                        if DBG:
                            nc.sync.dma_start(dbg_y.ap()[st, hl * P:(hl + 1) * P, :],
                                              y_h[:, st * SSH:(st + 1) * SSH])

            nc.sync.dma_start(ssq_in.rearrange("(a p) -> p a", p=P), ssq_acc[:])
            if DBG:
                nc.sync.dma_start(dbg_ssq.ap().rearrange("(a p) -> p a", p=P), ssq_acc[:])

            # ================= collectives ======================================
            if PHASES == "att":
                return _finish(nc)
            NOCOLL = bool(int(os.environ.get("KERNEL_NOCOLL", "0")))
            if NOCOLL:
                a2a_out = a2a_in
                ssq_out = ssq_in[:SSH]
            else:
                nc.gpsimd.collective_compute(
                    "AllToAll", ALU.bypass, replica_groups=[list(range(W))],
                    ins=[a2a_in.opt()], outs=[a2a_out.opt()])
                nc.gpsimd.collective_compute(
                    "ReduceScatter", ALU.add, replica_groups=[list(range(W))],
                    ins=[ssq_in.opt()], outs=[ssq_out.opt()])

            # rsqrt(var + eps)
            sq_raw = const.tile([P, SSH // P], F32)
            nc.sync.dma_start(sq_raw[:], ssq_out.rearrange("(i p) -> p i", p=P))
            t1 = const.tile([P, SSH // P], F32)
            nc.scalar.activation(t1[:], sq_raw[:], AF.Sqrt, bias=eps_t[:], scale=1.0 / INNER)
            nc.vector.reciprocal(rsq[:], t1[:])
            if DBG:
                nc.sync.dma_start(dbg_rsq.ap()[:], rsq[:])

            # ================= out projection (fp32r, seq-sharded) ==============
            with tc.tile_pool(name="oa", bufs=1) as oa, \
                 tc.tile_pool(name="ow", bufs=5) as ow, \
                 tc.tile_pool(name="oo", bufs=4) as oo, \
                 tc.tile_pool(name="op", bufs=4, space="PSUM") as opp:
                a2a_t = oa.tile([P, KO, SSH], BF16)
                a2a_v = a2a_out.rearrange("r (jo p) s -> p (r jo) s", p=P)
                for q in range(4):
                    nc.sync.dma_start(a2a_t[:, q * (KO // 4):(q + 1) * (KO // 4), :],
                                      a2a_v[:, q * (KO // 4):(q + 1) * (KO // 4), :])
                if DBG:
                    nc.sync.dma_start(dbg_a2a.ap().rearrange("r (jo p) s -> p (r jo) s", p=P), a2a_t[:])
                woutT_v = woutT.ap().rearrange("(ko p) o -> p ko o", p=P)
                out_v = out.ap().rearrange("(mo p) o -> p mo o", p=P)
                KC = KO // 4   # 8 k-subtiles per weight chunk
                for nt in range(H // 512):
                    osl = slice(nt * 512, (nt + 1) * 512)
                    w_ts = []
                    for q in range(4):
                        w_t = ow.tile([P, KC, 512], BF16, tag="w")
                        nc.sync.dma_start(w_t[:], woutT_v[:, q * KC:(q + 1) * KC, osl])
                        w_ts.append(w_t)
                    for mt in range(SSH // P):
                        ps = opp.tile([P, 512], F32, tag="po")
                        for kk in range(KO):
                            nc.tensor.matmul(ps[:], a2a_t[:, kk, mt * P:(mt + 1) * P],
                                             w_ts[kk // KC][:, kk % KC, :],
                                             start=(kk == 0), stop=(kk == KO - 1))
                        o_sb = oo.tile([P, 512], F32, tag="ot")
                        nc.vector.tensor_mul(o_sb[:], ps[:],
                                             rsq[:, mt:mt + 1].to_broadcast([P, 512]))
                        nc.sync.dma_start(out_v[:, mt, osl], o_sb[:])

    nc.compile()
    return nc


def _host_prep(inputs):
    x = np.asarray(inputs["x"], np.float32)
    w_qkv = np.asarray(inputs["w_qkv"], np.float32)
    w_gate = np.asarray(inputs["w_gate"], np.float32)
    w_out = np.asarray(inputs["w_out"], np.float32)
    norm_weight = np.asarray(inputs["norm_weight"], np.float32)
    kv_cache = np.asarray(inputs["kv_cache"], np.float32)
    slope = np.asarray(inputs["slope"], np.float32)

    bf = ml_dtypes.bfloat16
    xT_bf = np.ascontiguousarray(x.T).astype(bf)
    woutT = np.ascontiguousarray((w_out * norm_weight[None, :]).T).astype(bf)
    ident = np.eye(P, dtype=np.float32)
    ones = np.ones((P, P), ml_dtypes.bfloat16)

    in_maps = []
    for c in range(W):
        sl = slope[c * HPC:(c + 1) * HPC]                     # [4]
        m0 = np.arange(BLOCK, dtype=np.float32)              # 0-based position in block
        # qdec[p, hl, m] = exp(-s*(m+1)) replicated over partitions
        qd = np.exp(-sl[:, None] * (m0[None, :] + 1.0))      # [4, 256]
        qdec_a = np.broadcast_to(qd[None], (P, HPC, BLOCK)).astype(np.float32).copy()
        # kdec[p, hl, no] = exp(-s*(BLOCK - (no*128+p+1)))
        n0 = (np.arange(2)[None, :] * P + np.arange(P)[:, None]).astype(np.float32)  # [128,2]
        kd = np.exp(-sl[None, None, :] * (BLOCK - (n0[:, :, None] + 1.0)))           # [128,2,4]
        kdec_a = np.ascontiguousarray(kd.transpose(0, 2, 1)).astype(np.float32)      # [128,4,2]
        # maskT[p, hl, no, m] = exp(-s*(m - n)) if m>=n else 0   (0-based n = no*128+p)
        nfull = n0[:, :, None]                                # [128,2,1]
        diff = m0[None, None, :] - nfull                      # [128,2,256]
        dif4 = diff[..., None]                                # [128,2,256,1]
        mask = np.where(dif4 >= 0,
                        np.exp(-sl[None, None, None, :] * np.maximum(dif4, 0.0)),
                        0.0)                                  # [128,2,256,4]
        maskT_a = np.ascontiguousarray(mask.transpose(0, 3, 1, 2)).astype(np.float32)        # [128,4,2,256]
        blkdec_a = np.broadcast_to(np.exp(-sl * BLOCK)[None], (P, HPC)).astype(np.float32).copy()

        in_maps.append({
            "xT": xT_bf,
            "wqkvT": np.ascontiguousarray(w_qkv[MPC * c:MPC * (c + 1)].T).astype(bf),
            "wgateT": np.ascontiguousarray(w_gate[JPC * c:JPC * (c + 1)].T).astype(bf),
            "woutT": woutT,
            "qdec": qdec_a,
            "kdec": kdec_a,
            "maskT": maskT_a,
            "blkdec": blkdec_a,
            "ident_r": ident,
            "ones_r": ones,
            "eps_b": np.full((P, 1), EPS, np.float32),
            "kv0": np.ascontiguousarray(kv_cache[HPC * c:HPC * (c + 1)]),
        })
    return in_maps


_CACHE = {}


def _get_program():
    if "nc" not in _CACHE:
        _CACHE["nc"] = _build_program()
    return _CACHE["nc"]


def kernel(**inputs):
    nc = _get_program()
    in_maps = _host_prep(inputs)
    trace = bool(int(os.environ.get("KERNEL_TRACE", "0")))
    res = run_bass_kernel_spmd(nc, in_maps, core_ids=list(range(W)), trace=trace)
    _CACHE["last_results"] = res
    out = np.concatenate([res.results[c]["out"] for c in range(W)], axis=0)
    return out.astype(np.float32)


# revision 23
# speedup vs baseline: 1.1090x; 1.0082x over previous
"""MiniMaxText01 linear attention (lightning attention) prefill layer on 8 TRN2 NeuronCores.

Strategy: tensor-parallel over heads (4 heads/core) for qkv+gate+attention;
AllToAll to switch to sequence-parallel for the output projection;
ReduceScatter for the RMSNorm variance. See spec sharding_hint.
"""
import os
import sys
import math

sys.path.insert(0, "/opt/trn_rl_repo")

import numpy as np
import ml_dtypes

import concourse.bass as bass
import concourse.mybir as mybir
import concourse.tile as tile
from concourse import bacc
from concourse.bass_utils import run_bass_kernel_spmd

# problem constants (hardcoded per contract)
H = 4096
INNER = 4096
NH = 32
HD = 128
SEQ = 4096
BLOCK = 256
EPS = 1e-5
P = 128
W = 8                    # cores
HPC = NH // W            # heads per core = 4
MPC = 3 * HD * HPC       # qkv rows per core = 1536
JPC = HD * HPC           # inner cols per core = 512
SSH = SEQ // W           # seq shard = 512
KO = H // P              # 32 k-subtiles
NB = SEQ // BLOCK        # 16 blocks
SB = SEQ // P            # 32 sub-blocks of 128

F32 = mybir.dt.float32
F32R = mybir.dt.float32r
BF16 = mybir.dt.bfloat16
AF = mybir.ActivationFunctionType
ALU = mybir.AluOpType


def _finish(nc):
    return nc


def _build_program():
    nc = bacc.Bacc("TRN2", target_bir_lowering=False, debug=False, num_devices=W)

    # ---- I/O ----
    xT = nc.dram_tensor("xT", [H, SEQ], BF16, kind="ExternalInput")
    wqkvT = nc.dram_tensor("wqkvT", [H, MPC], BF16, kind="ExternalInput")
    wgateT = nc.dram_tensor("wgateT", [H, JPC], BF16, kind="ExternalInput")
    woutT = nc.dram_tensor("woutT", [INNER, H], BF16, kind="ExternalInput")
    qdec = nc.dram_tensor("qdec", [P, HPC, BLOCK], F32, kind="ExternalInput")
    kdec = nc.dram_tensor("kdec", [P, HPC, 2], F32, kind="ExternalInput")
    maskT = nc.dram_tensor("maskT", [P, HPC, 2, BLOCK], F32, kind="ExternalInput")
    blkdec = nc.dram_tensor("blkdec", [P, HPC], F32, kind="ExternalInput")
    ident_r = nc.dram_tensor("ident_r", [P, P], F32R, kind="ExternalInput")
    ones_r = nc.dram_tensor("ones_r", [P, P], BF16, kind="ExternalInput")
    eps_b = nc.dram_tensor("eps_b", [P, 1], F32, kind="ExternalInput")
    kv0 = nc.dram_tensor("kv0", [HPC, HD, HD], F32, kind="ExternalInput")
    out = nc.dram_tensor("out", [SSH, H], F32, kind="ExternalOutput")
    DBG = bool(int(os.environ.get("KERNEL_DEBUG", "0")))
    if DBG:
        dbg_qkvT = nc.dram_tensor("dbg_qkvT", [MPC, SEQ], F32R, kind="ExternalOutput")
        dbg_gateT = nc.dram_tensor("dbg_gateT", [JPC, SEQ], F32, kind="ExternalOutput")
        dbg_hidT = nc.dram_tensor("dbg_hidT", [JPC, SEQ], F32, kind="ExternalOutput")
        dbg_y = nc.dram_tensor("dbg_y", [W, JPC, SSH], BF16, kind="ExternalOutput")
        dbg_a2a = nc.dram_tensor("dbg_a2a", [W, JPC, SSH], BF16, kind="ExternalOutput")
        dbg_ssq = nc.dram_tensor("dbg_ssq", [SEQ], F32, kind="ExternalOutput")
        dbg_rsq = nc.dram_tensor("dbg_rsq", [P, SSH // P], F32, kind="ExternalOutput")

    with tile.TileContext(nc) as tc:
        with tc.tile_pool(name="dram", bufs=1, space="DRAM") as dram, \
             tc.tile_pool(name="const", bufs=1) as const:
            # ---- DRAM temporaries ----
            qkvT_d0 = dram.tile([MPC // 2, SEQ], F32R)           # silu(qkv) heads 0-1, transposed
            qkvT_d1 = dram.tile([MPC // 2, SEQ], F32R)           # heads 2-3
            gateT_d = dram.tile([JPC, SEQ], F32)                 # sigmoid gate, transposed
            a2a_in = dram.tile([W, JPC, SSH], BF16)              # Y shards (j-major per shard)
            a2a_out = dram.tile([W, JPC, SSH], BF16)
            ssq_in = dram.tile([SEQ], F32)
            ssq_out = dram.tile([SSH], F32)

            # ---- constants in SBUF ----
            qdec_t = const.tile([P, HPC, BLOCK], F32)
            nc.sync.dma_start(qdec_t[:], qdec.ap()[:])
            kdec_t = const.tile([P, HPC, 2], F32)
            nc.sync.dma_start(kdec_t[:], kdec.ap()[:])
            maskT_t = const.tile([P, HPC, 2, BLOCK], F32)
            nc.sync.dma_start(maskT_t[:], maskT.ap()[:])
            blkdec_t = const.tile([P, HPC], F32)
            nc.sync.dma_start(blkdec_t[:], blkdec.ap()[:])
            ident_t = const.tile([P, P], F32R)
            nc.sync.dma_start(ident_t[:], ident_r.ap()[:])
            ones_t = const.tile([P, P], BF16)
            nc.sync.dma_start(ones_t[:], ones_r.ap()[:])
            eps_t = const.tile([P, 1], F32)
            nc.sync.dma_start(eps_t[:], eps_b.ap()[:])
            ssq_acc = const.tile([P, SB], F32)
            rsq = const.tile([P, SSH // P], F32)

            xT_v = xT.ap().rearrange("(ko p) s -> p ko s", p=P)
            wqkvT_v = wqkvT.ap().rearrange("(ko p) m -> p ko m", p=P)
            wgateT_v = wgateT.ap().rearrange("(ko p) m -> p ko m", p=P)
            qkvT_v0 = qkvT_d0.rearrange("(mo p) s -> p mo s", p=P)
            qkvT_v1 = qkvT_d1.rearrange("(mo p) s -> p mo s", p=P)
            gateT_v = gateT_d.rearrange("(go p) s -> p go s", p=P)

            # ================= phase AB: qkv + gate projections (bf16) ==========
            MG = MPC // 2   # 768 cols of wqkvT per group
            KC = 8          # k-subtiles per chunk (4 chunks cover K)
            NKC = KO // KC
            with tc.tile_pool(name="abw", bufs=NKC + 1) as wp, \
                 tc.tile_pool(name="abg", bufs=NKC) as gp, \
                 tc.tile_pool(name="abx", bufs=NKC + 2) as xp, \
                 tc.tile_pool(name="abo", bufs=4) as op, \
                 tc.tile_pool(name="abp", bufs=4, space="PSUM") as pp:
                wg_c = []
                for kc in range(NKC):
                    wg_t = gp.tile([P, KC, JPC], BF16, tag="wg", name=f"wg{kc}")
                    nc.sync.dma_start(wg_t[:], wgateT_v[:, kc * KC:(kc + 1) * KC, :])
                    wg_c.append(wg_t)
                for grp in range(2):
                    wq_c = []
                    for kc in range(NKC):
                        wq_t = wp.tile([P, KC, MG], BF16, tag="wq", name=f"wq{grp}_{kc}")
                        nc.sync.dma_start(wq_t[:], wqkvT_v[:, kc * KC:(kc + 1) * KC,
                                                           grp * MG:(grp + 1) * MG])
                        wq_c.append(wq_t)
                    for n in range(SEQ // 512):
                        x_c = []
                        for kc in range(NKC):
                            x_t = xp.tile([P, KC, 512], BF16, tag="x", name=f"x{grp}_{n}_{kc}")
                            nc.sync.dma_start(x_t[:], xT_v[:, kc * KC:(kc + 1) * KC,
                                                           n * 512:(n + 1) * 512])
                            x_c.append(x_t)
                        for mm in range(MG // P):
                            ps = pp.tile([P, 512], F32, tag="ps")
                            for k in range(KO):
                                nc.tensor.matmul(ps[:], wq_c[k // KC][:, k % KC, mm * P:(mm + 1) * P],
                                                 x_c[k // KC][:, k % KC, :],
                                                 start=(k == 0), stop=(k == KO - 1))
                            o_t = op.tile([P, 512], F32R, tag="o")
                            nc.scalar.activation(o_t[:], ps[:], AF.Silu)
                            qv = qkvT_v0 if grp == 0 else qkvT_v1
                            nc.sync.dma_start(qv[:, mm, n * 512:(n + 1) * 512], o_t[:])
                            if DBG:
                                m_global = grp * (MG // P) + mm
                                nc.sync.dma_start(
                                    dbg_qkvT.ap().rearrange("(mo p) s -> p mo s", p=P)[:, m_global, n * 512:(n + 1) * 512], o_t[:])
                        if grp == 0:
                            for gg in range(JPC // P):
                                ps = pp.tile([P, 512], F32, tag="ps")
                                for k in range(KO):
                                    nc.tensor.matmul(ps[:], wg_c[k // KC][:, k % KC, gg * P:(gg + 1) * P],
                                                     x_c[k // KC][:, k % KC, :],
                                                     start=(k == 0), stop=(k == KO - 1))
                                g_t = op.tile([P, 512], F32, tag="g")
                                nc.scalar.activation(g_t[:], ps[:], AF.Sigmoid)
                                nc.sync.dma_start(gateT_v[:, gg, n * 512:(n + 1) * 512], g_t[:])
                                if DBG:
                                    nc.sync.dma_start(
                                        dbg_gateT.ap().rearrange("(go p) s -> p go s", p=P)[:, gg, n * 512:(n + 1) * 512], g_t[:])

            PHASES = os.environ.get("KERNEL_PHASES", "full")
            # ================= attention (fp32r) per head =======================
            if PHASES == "ab":
                return _finish(nc)
            with tc.tile_pool(name="atth", bufs=1) as ah, \
                 tc.tile_pool(name="atts", bufs=3) as asml, \
                 tc.tile_pool(name="attp", bufs=2, space="PSUM") as ap_:
                for hl in range(HPC):
                    qv = qkvT_v0 if hl < 2 else qkvT_v1
                    mo0 = 3 * (hl % 2)
                    gate_h = ah.tile([P, SEQ], F32, tag="gate", bufs=2)
                    nc.sync.dma_start(gate_h[:], gateT_v[:, hl, :])
                    kv_state = ah.tile([P, HD], F32, tag="kv")
                    nc.sync.dma_start(kv_state[:], kv0.ap()[hl])
                    hiddenT_h = ah.tile([P, SEQ], F32, tag="hid", bufs=2)
                    # kv_all[:, b] = kv state BEFORE block b; updated one block ahead
                    # of its consumer so the o-matmuls never wait on the recurrence.
                    kv_all = ah.tile([P, NB, HD], F32R, tag="kva", bufs=2)
                    nc.scalar.activation(kv_all[:, 0, :], kv_state[:], AF.Copy)

                    for b in range(NB):
                        msl = slice(b * BLOCK, (b + 1) * BLOCK)
                        qkv_c = asml.tile([P, 3, BLOCK], F32R, tag="qkv_c", bufs=6)
                        nc.sync.dma_start(qkv_c[:], qv[:, mo0:mo0 + 3, msl])
                        kdvn = asml.tile([P, 2, 2, HD], F32R, tag="kdvn", bufs=6)
                        for no in range(2):
                            pst = ap_.tile([P, P], F32R, tag="tr")
                            nc.tensor.transpose(pst[:], qkv_c[:, 1, no * P:(no + 1) * P], ident_t[:])
                            nc.scalar.activation(kdvn[:, 0, no, :], pst[:], AF.Copy,
                                                 scale=kdec_t[:, hl, no:no + 1])
                            pst2 = ap_.tile([P, P], F32R, tag="tr")
                            nc.tensor.transpose(pst2[:], qkv_c[:, 2, no * P:(no + 1) * P], ident_t[:])
                            nc.any.tensor_copy(kdvn[:, 1, no, :], pst2[:])

                        if b < NB - 1:
                            c_ps = ap_.tile([P, HD], F32, tag="C")
                            nc.tensor.matmul(c_ps[:], kdvn[:, 0, 0, :], kdvn[:, 1, 0, :], start=True, stop=False)
                            nc.tensor.matmul(c_ps[:], kdvn[:, 0, 1, :], kdvn[:, 1, 1, :], start=False, stop=True)
                            nc.vector.tensor_mul(kv_all[:, b + 1, :], kv_all[:, b, :],
                                                 blkdec_t[:, hl:hl + 1].to_broadcast([P, HD]))
                            nc.vector.tensor_add(kv_all[:, b + 1, :], kv_all[:, b + 1, :], c_ps[:])

                        qk_ps = []
                        for no in range(2):
                            qk = ap_.tile([P, BLOCK], F32, tag="qk")
                            nc.tensor.matmul(qk[:], qkv_c[:, 1, no * P:(no + 1) * P],
                                             qkv_c[:, 0, :], start=True, stop=True)
                            qk_ps.append(qk)
                        qkTm = asml.tile([P, 2, BLOCK], F32R, tag="qkm")
                        for no in range(2):
                            nc.vector.tensor_mul(qkTm[:, no, :], qk_ps[no][:], maskT_t[:, hl, no, :])
                        qdT = asml.tile([P, BLOCK], F32R, tag="qdT")
                        nc.vector.tensor_mul(qdT[:], qkv_c[:, 0, :], qdec_t[:, hl, :])

                        o_ps = ap_.tile([P, BLOCK], F32, tag="o")
                        nc.tensor.matmul(o_ps[:], kdvn[:, 1, 0, :], qkTm[:, 0, :], start=True, stop=False)
                        nc.tensor.matmul(o_ps[:], kdvn[:, 1, 1, :], qkTm[:, 1, :], start=False, stop=False)
                        nc.tensor.matmul(o_ps[:], kv_all[:, b, :], qdT[:], start=False, stop=True)
                        nc.any.tensor_copy(hiddenT_h[:, msl], o_ps[:])

                    # ssq partial + gated Y
                    sq_h = ah.tile([P, SEQ], BF16, tag="sq")
                    nc.vector.tensor_mul(sq_h[:], hiddenT_h[:], hiddenT_h[:])
                    for sc in range(SB):
                        sp = ap_.tile([P, P], F32, tag="C")
                        nc.tensor.matmul(sp[:], sq_h[:, sc * P:(sc + 1) * P], ones_t[:],
                                         start=True, stop=True)
                        if hl == 0:
                            nc.vector.tensor_copy(ssq_acc[:, sc:sc + 1], sp[:, 0:1])
                        else:
                            nc.vector.tensor_add(ssq_acc[:, sc:sc + 1], ssq_acc[:, sc:sc + 1], sp[:, 0:1])
                    if DBG:
                        nc.sync.dma_start(
                            dbg_hidT.ap().rearrange("(h p) s -> p h s", p=P)[:, hl, :], hiddenT_h[:])
                    y_h = ah.tile([P, SEQ], BF16, tag="y")
                    nc.vector.tensor_mul(y_h[:], hiddenT_h[:], gate_h[:])
                    for st in range(W):
                        nc.sync.dma_start(a2a_in[st, hl * P:(hl + 1) * P, :],
                                          y_h[:, st * SSH:(st + 1) * SSH])
                        if DBG:
                            nc.sync.dma_start(dbg_y.ap()[st, hl * P:(hl + 1) * P, :],
                                              y_h[:, st * SSH:(st + 1) * SSH])

            nc.sync.dma_start(ssq_in.rearrange("(a p) -> p a", p=P), ssq_acc[:])
            if DBG:
                nc.sync.dma_start(dbg_ssq.ap().rearrange("(a p) -> p a", p=P), ssq_acc[:])

            # ================= collectives ======================================
            if PHASES == "att":
                return _finish(nc)
            NOCOLL = bool(int(os.environ.get("KERNEL_NOCOLL", "0")))
            if NOCOLL:
                a2a_out = a2a_in
                ssq_out = ssq_in[:SSH]
            else:
                nc.gpsimd.collective_compute(
                    "AllToAll", ALU.bypass, replica_groups=[list(range(W))],
                    ins=[a2a_in.opt()], outs=[a2a_out.opt()])
                nc.gpsimd.collective_compute(
                    "ReduceScatter", ALU.add, replica_groups=[list(range(W))],
                    ins=[ssq_in.opt()], outs=[ssq_out.opt()])

            # rsqrt(var + eps)
            sq_raw = const.tile([P, SSH // P], F32)
            nc.sync.dma_start(sq_raw[:], ssq_out.rearrange("(i p) -> p i", p=P))
            t1 = const.tile([P, SSH // P], F32)
            nc.scalar.activation(t1[:], sq_raw[:], AF.Sqrt, bias=eps_t[:], scale=1.0 / INNER)
            nc.vector.reciprocal(rsq[:], t1[:])
            if DBG:
                nc.sync.dma_start(dbg_rsq.ap()[:], rsq[:])

            # ================= out projection (fp32r, seq-sharded) ==============
            with tc.tile_pool(name="oa", bufs=1) as oa, \
                 tc.tile_pool(name="ow", bufs=5) as ow, \
                 tc.tile_pool(name="oo", bufs=4) as oo, \
                 tc.tile_pool(name="op", bufs=4, space="PSUM") as opp:
                a2a_t = oa.tile([P, KO, SSH], BF16)
                a2a_v = a2a_out.rearrange("r (jo p) s -> p (r jo) s", p=P)
                for q in range(4):
                    nc.sync.dma_start(a2a_t[:, q * (KO // 4):(q + 1) * (KO // 4), :],
                                      a2a_v[:, q * (KO // 4):(q + 1) * (KO // 4), :])
                if DBG:
                    nc.sync.dma_start(dbg_a2a.ap().rearrange("r (jo p) s -> p (r jo) s", p=P), a2a_t[:])
                woutT_v = woutT.ap().rearrange("(ko p) o -> p ko o", p=P)
                out_v = out.ap().rearrange("(mo p) o -> p mo o", p=P)
                KC = KO // 4   # 8 k-subtiles per weight chunk
                for nt in range(H // 512):
                    osl = slice(nt * 512, (nt + 1) * 512)
                    w_ts = []
                    for q in range(4):
                        w_t = ow.tile([P, KC, 512], BF16, tag="w")
                        nc.sync.dma_start(w_t[:], woutT_v[:, q * KC:(q + 1) * KC, osl])
                        w_ts.append(w_t)
                    for mt in range(SSH // P):
                        ps = opp.tile([P, 512], F32, tag="po")
                        for kk in range(KO):
                            nc.tensor.matmul(ps[:], a2a_t[:, kk, mt * P:(mt + 1) * P],
                                             w_ts[kk // KC][:, kk % KC, :],
                                             start=(kk == 0), stop=(kk == KO - 1))
                        o_sb = oo.tile([P, 512], F32, tag="ot")
                        nc.vector.tensor_mul(o_sb[:], ps[:],
                                             rsq[:, mt:mt + 1].to_broadcast([P, 512]))
                        nc.sync.dma_start(out_v[:, mt, osl], o_sb[:])

    nc.compile()
    return nc


def _host_prep(inputs):
    x = np.asarray(inputs["x"], np.float32)
    w_qkv = np.asarray(inputs["w_qkv"], np.float32)
    w_gate = np.asarray(inputs["w_gate"], np.float32)
    w_out = np.asarray(inputs["w_out"], np.float32)
    norm_weight = np.asarray(inputs["norm_weight"], np.float32)
    kv_cache = np.asarray(inputs["kv_cache"], np.float32)
    slope = np.asarray(inputs["slope"], np.float32)

    bf = ml_dtypes.bfloat16
    xT_bf = np.ascontiguousarray(x.T).astype(bf)
    woutT = np.ascontiguousarray((w_out * norm_weight[None, :]).T).astype(bf)
    ident = np.eye(P, dtype=np.float32)
    ones = np.ones((P, P), ml_dtypes.bfloat16)

    in_maps = []
    for c in range(W):
        sl = slope[c * HPC:(c + 1) * HPC]                     # [4]
        m0 = np.arange(BLOCK, dtype=np.float32)              # 0-based position in block
        # qdec[p, hl, m] = exp(-s*(m+1)) replicated over partitions
        qd = np.exp(-sl[:, None] * (m0[None, :] + 1.0))      # [4, 256]
        qdec_a = np.broadcast_to(qd[None], (P, HPC, BLOCK)).astype(np.float32).copy()
        # kdec[p, hl, no] = exp(-s*(BLOCK - (no*128+p+1)))
        n0 = (np.arange(2)[None, :] * P + np.arange(P)[:, None]).astype(np.float32)  # [128,2]
        kd = np.exp(-sl[None, None, :] * (BLOCK - (n0[:, :, None] + 1.0)))           # [128,2,4]
        kdec_a = np.ascontiguousarray(kd.transpose(0, 2, 1)).astype(np.float32)      # [128,4,2]
        # maskT[p, hl, no, m] = exp(-s*(m - n)) if m>=n else 0   (0-based n = no*128+p)
        nfull = n0[:, :, None]                                # [128,2,1]
        diff = m0[None, None, :] - nfull                      # [128,2,256]
        dif4 = diff[..., None]                                # [128,2,256,1]
        mask = np.where(dif4 >= 0,
                        np.exp(-sl[None, None, None, :] * np.maximum(dif4, 0.0)),
                        0.0)                                  # [128,2,256,4]
        maskT_a = np.ascontiguousarray(mask.transpose(0, 3, 1, 2)).astype(np.float32)        # [128,4,2,256]
        blkdec_a = np.broadcast_to(np.exp(-sl * BLOCK)[None], (P, HPC)).astype(np.float32).copy()

        in_maps.append({
            "xT": xT_bf,
            "wqkvT": np.ascontiguousarray(w_qkv[MPC * c:MPC * (c + 1)].T).astype(bf),
            "wgateT": np.ascontiguousarray(w_gate[JPC * c:JPC * (c + 1)].T).astype(bf),
            "woutT": woutT,
            "qdec": qdec_a,
            "kdec": kdec_a,
            "maskT": maskT_a,
            "blkdec": blkdec_a,
            "ident_r": ident,
            "ones_r": ones,
            "eps_b": np.full((P, 1), EPS, np.float32),
            "kv0": np.ascontiguousarray(kv_cache[HPC * c:HPC * (c + 1)]),
        })
    return in_maps


_CACHE = {}


def _get_program():
    if "nc" not in _CACHE:
        _CACHE["nc"] = _build_program()
    return _CACHE["nc"]


def kernel(**inputs):
    nc = _get_program()
    in_maps = _host_prep(inputs)
    trace = bool(int(os.environ.get("KERNEL_TRACE", "0")))
    res = run_bass_kernel_spmd(nc, in_maps, core_ids=list(range(W)), trace=trace)
    _CACHE["last_results"] = res
    out = np.concatenate([res.results[c]["out"] for c in range(W)], axis=0)
    return out.astype(np.float32)


# revision 24
# speedup vs baseline: 1.1140x; 1.0045x over previous
"""MiniMaxText01 linear attention (lightning attention) prefill layer on 8 TRN2 NeuronCores.

Strategy: tensor-parallel over heads (4 heads/core) for qkv+gate+attention;
AllToAll to switch to sequence-parallel for the output projection;
ReduceScatter for the RMSNorm variance. See spec sharding_hint.
"""
import os
import sys
import math

sys.path.insert(0, "/opt/trn_rl_repo")

import numpy as np
import ml_dtypes

import concourse.bass as bass
import concourse.mybir as mybir
import concourse.tile as tile
from concourse import bacc
from concourse.bass_utils import run_bass_kernel_spmd

# problem constants (hardcoded per contract)
H = 4096
INNER = 4096
NH = 32
HD = 128
SEQ = 4096
BLOCK = 256
EPS = 1e-5
P = 128
W = 8                    # cores
HPC = NH // W            # heads per core = 4
MPC = 3 * HD * HPC       # qkv rows per core = 1536
JPC = HD * HPC           # inner cols per core = 512
SSH = SEQ // W           # seq shard = 512
KO = H // P              # 32 k-subtiles
NB = SEQ // BLOCK        # 16 blocks
SB = SEQ // P            # 32 sub-blocks of 128

F32 = mybir.dt.float32
F32R = mybir.dt.float32r
BF16 = mybir.dt.bfloat16
AF = mybir.ActivationFunctionType
ALU = mybir.AluOpType


def _finish(nc):
    return nc


def _build_program():
    nc = bacc.Bacc("TRN2", target_bir_lowering=False, debug=False, num_devices=W)

    # ---- I/O ----
    xT = nc.dram_tensor("xT", [H, SEQ], BF16, kind="ExternalInput")
    wqkvT = nc.dram_tensor("wqkvT", [H, MPC], BF16, kind="ExternalInput")
    wgateT = nc.dram_tensor("wgateT", [H, JPC], BF16, kind="ExternalInput")
    woutT = nc.dram_tensor("woutT", [INNER, H], BF16, kind="ExternalInput")
    qdec = nc.dram_tensor("qdec", [P, HPC, BLOCK], F32, kind="ExternalInput")
    kdec = nc.dram_tensor("kdec", [P, HPC, 2], F32, kind="ExternalInput")
    maskT = nc.dram_tensor("maskT", [P, HPC, 2, BLOCK], F32, kind="ExternalInput")
    blkdec = nc.dram_tensor("blkdec", [P, HPC], F32, kind="ExternalInput")
    ident_r = nc.dram_tensor("ident_r", [P, P], F32R, kind="ExternalInput")
    ones_r = nc.dram_tensor("ones_r", [P, P], BF16, kind="ExternalInput")
    eps_b = nc.dram_tensor("eps_b", [P, 1], F32, kind="ExternalInput")
    kv0 = nc.dram_tensor("kv0", [HPC, HD, HD], F32, kind="ExternalInput")
    out = nc.dram_tensor("out", [SSH, H], F32, kind="ExternalOutput")
    DBG = bool(int(os.environ.get("KERNEL_DEBUG", "0")))
    if DBG:
        dbg_qkvT = nc.dram_tensor("dbg_qkvT", [MPC, SEQ], F32R, kind="ExternalOutput")
        dbg_gateT = nc.dram_tensor("dbg_gateT", [JPC, SEQ], F32, kind="ExternalOutput")
        dbg_hidT = nc.dram_tensor("dbg_hidT", [JPC, SEQ], F32, kind="ExternalOutput")
        dbg_y = nc.dram_tensor("dbg_y", [W, JPC, SSH], BF16, kind="ExternalOutput")
        dbg_a2a = nc.dram_tensor("dbg_a2a", [W, JPC, SSH], BF16, kind="ExternalOutput")
        dbg_ssq = nc.dram_tensor("dbg_ssq", [SEQ], F32, kind="ExternalOutput")
        dbg_rsq = nc.dram_tensor("dbg_rsq", [P, SSH // P], F32, kind="ExternalOutput")

    with tile.TileContext(nc) as tc:
        with tc.tile_pool(name="dram", bufs=1, space="DRAM") as dram, \
             tc.tile_pool(name="const", bufs=1) as const:
            # ---- DRAM temporaries ----
            qkvT_d0 = dram.tile([MPC // 2, SEQ], F32R)           # silu(qkv) heads 0-1, transposed
            qkvT_d1 = dram.tile([MPC // 2, SEQ], F32R)           # heads 2-3
            gateT_d = dram.tile([JPC, SEQ], F32)                 # sigmoid gate, transposed
            a2a_in = dram.tile([W, JPC, SSH], BF16)              # Y shards (j-major per shard)
            a2a_out = dram.tile([W, JPC, SSH], BF16)
            ssq_in = dram.tile([SEQ], F32)
            ssq_out = dram.tile([SSH], F32)

            # ---- constants in SBUF ----
            qdec_t = const.tile([P, HPC, BLOCK], F32)
            nc.sync.dma_start(qdec_t[:], qdec.ap()[:])
            kdec_t = const.tile([P, HPC, 2], F32)
            nc.sync.dma_start(kdec_t[:], kdec.ap()[:])
            maskT_t = const.tile([P, HPC, 2, BLOCK], F32)
            nc.sync.dma_start(maskT_t[:], maskT.ap()[:])
            blkdec_t = const.tile([P, HPC], F32)
            nc.sync.dma_start(blkdec_t[:], blkdec.ap()[:])
            ident_t = const.tile([P, P], F32R)
            nc.sync.dma_start(ident_t[:], ident_r.ap()[:])
            ones_t = const.tile([P, P], BF16)
            nc.sync.dma_start(ones_t[:], ones_r.ap()[:])
            eps_t = const.tile([P, 1], F32)
            nc.sync.dma_start(eps_t[:], eps_b.ap()[:])
            ssq_acc = const.tile([P, SB], F32)
            rsq = const.tile([P, SSH // P], F32)

            xT_v = xT.ap().rearrange("(ko p) s -> p ko s", p=P)
            wqkvT_v = wqkvT.ap().rearrange("(ko p) m -> p ko m", p=P)
            wgateT_v = wgateT.ap().rearrange("(ko p) m -> p ko m", p=P)
            qkvT_v0 = qkvT_d0.rearrange("(mo p) s -> p mo s", p=P)
            qkvT_v1 = qkvT_d1.rearrange("(mo p) s -> p mo s", p=P)
            gateT_v = gateT_d.rearrange("(go p) s -> p go s", p=P)

            # ================= phase AB: qkv + gate projections (bf16) ==========
            MG = MPC // 2   # 768 cols of wqkvT per group
            KC = 8          # k-subtiles per chunk (4 chunks cover K)
            NKC = KO // KC
            with tc.tile_pool(name="abw", bufs=NKC + 1) as wp, \
                 tc.tile_pool(name="abg", bufs=NKC) as gp, \
                 tc.tile_pool(name="abx", bufs=NKC + 2) as xp, \
                 tc.tile_pool(name="abo", bufs=4) as op, \
                 tc.tile_pool(name="abp", bufs=4, space="PSUM") as pp:
                wg_c = []
                for kc in range(NKC):
                    wg_t = gp.tile([P, KC, JPC], BF16, tag="wg", name=f"wg{kc}")
                    nc.sync.dma_start(wg_t[:], wgateT_v[:, kc * KC:(kc + 1) * KC, :])
                    wg_c.append(wg_t)
                for grp in range(2):
                    wq_c = []
                    for kc in range(NKC):
                        wq_t = wp.tile([P, KC, MG], BF16, tag="wq", name=f"wq{grp}_{kc}")
                        nc.sync.dma_start(wq_t[:], wqkvT_v[:, kc * KC:(kc + 1) * KC,
                                                           grp * MG:(grp + 1) * MG])
                        wq_c.append(wq_t)
                    for n in range(SEQ // 512):
                        x_c = []
                        for kc in range(NKC):
                            x_t = xp.tile([P, KC, 512], BF16, tag="x", name=f"x{grp}_{n}_{kc}")
                            nc.sync.dma_start(x_t[:], xT_v[:, kc * KC:(kc + 1) * KC,
                                                           n * 512:(n + 1) * 512])
                            x_c.append(x_t)
                        for mm in range(MG // P):
                            ps = pp.tile([P, 512], F32, tag="ps")
                            for k in range(KO):
                                nc.tensor.matmul(ps[:], wq_c[k // KC][:, k % KC, mm * P:(mm + 1) * P],
                                                 x_c[k // KC][:, k % KC, :],
                                                 start=(k == 0), stop=(k == KO - 1))
                            o_t = op.tile([P, 512], F32R, tag="o")
                            nc.scalar.activation(o_t[:], ps[:], AF.Silu)
                            qv = qkvT_v0 if grp == 0 else qkvT_v1
                            nc.sync.dma_start(qv[:, mm, n * 512:(n + 1) * 512], o_t[:])
                            if DBG:
                                m_global = grp * (MG // P) + mm
                                nc.sync.dma_start(
                                    dbg_qkvT.ap().rearrange("(mo p) s -> p mo s", p=P)[:, m_global, n * 512:(n + 1) * 512], o_t[:])
                        if grp == 0:
                            for gg in range(JPC // P):
                                ps = pp.tile([P, 512], F32, tag="ps")
                                for k in range(KO):
                                    nc.tensor.matmul(ps[:], wg_c[k // KC][:, k % KC, gg * P:(gg + 1) * P],
                                                     x_c[k // KC][:, k % KC, :],
                                                     start=(k == 0), stop=(k == KO - 1))
                                g_t = op.tile([P, 512], F32, tag="g")
                                nc.scalar.activation(g_t[:], ps[:], AF.Sigmoid)
                                nc.sync.dma_start(gateT_v[:, gg, n * 512:(n + 1) * 512], g_t[:])
                                if DBG:
                                    nc.sync.dma_start(
                                        dbg_gateT.ap().rearrange("(go p) s -> p go s", p=P)[:, gg, n * 512:(n + 1) * 512], g_t[:])

            PHASES = os.environ.get("KERNEL_PHASES", "full")
            # ================= attention (fp32r) per head =======================
            if PHASES == "ab":
                return _finish(nc)
            with tc.tile_pool(name="atth", bufs=1) as ah, \
                 tc.tile_pool(name="atts", bufs=3) as asml, \
                 tc.tile_pool(name="attp", bufs=2, space="PSUM") as ap_:
                for hl in range(HPC):
                    qv = qkvT_v0 if hl < 2 else qkvT_v1
                    mo0 = 3 * (hl % 2)
                    gate_h = ah.tile([P, SEQ], F32, tag="gate", bufs=2)
                    nc.sync.dma_start(gate_h[:], gateT_v[:, hl, :])
                    kv_state = ah.tile([P, HD], F32, tag="kv")
                    nc.sync.dma_start(kv_state[:], kv0.ap()[hl])
                    hiddenT_h = ah.tile([P, SEQ], F32, tag="hid", bufs=2)
                    # kv_all[:, b] = kv state BEFORE block b; updated one block ahead
                    # of its consumer so the o-matmuls never wait on the recurrence.
                    kv_all = ah.tile([P, NB, HD], F32R, tag="kva", bufs=2)
                    nc.scalar.activation(kv_all[:, 0, :], kv_state[:], AF.Copy)

                    for b in range(NB):
                        msl = slice(b * BLOCK, (b + 1) * BLOCK)
                        qkv_c = asml.tile([P, 3, BLOCK], F32R, tag="qkv_c", bufs=6)
                        nc.sync.dma_start(qkv_c[:], qv[:, mo0:mo0 + 3, msl])
                        kdvn = asml.tile([P, 2, 2, HD], BF16, tag="kdvn", bufs=6)
                        for no in range(2):
                            pst = ap_.tile([P, P], F32R, tag="tr")
                            nc.tensor.transpose(pst[:], qkv_c[:, 1, no * P:(no + 1) * P], ident_t[:])
                            nc.scalar.activation(kdvn[:, 0, no, :], pst[:], AF.Copy,
                                                 scale=kdec_t[:, hl, no:no + 1])
                            pst2 = ap_.tile([P, P], F32R, tag="tr")
                            nc.tensor.transpose(pst2[:], qkv_c[:, 2, no * P:(no + 1) * P], ident_t[:])
                            nc.any.tensor_copy(kdvn[:, 1, no, :], pst2[:])

                        if b < NB - 1:
                            c_ps = ap_.tile([P, HD], F32, tag="C")
                            nc.tensor.matmul(c_ps[:], kdvn[:, 0, 0, :], kdvn[:, 1, 0, :], start=True, stop=False)
                            nc.tensor.matmul(c_ps[:], kdvn[:, 0, 1, :], kdvn[:, 1, 1, :], start=False, stop=True)
                            nc.vector.tensor_mul(kv_all[:, b + 1, :], kv_all[:, b, :],
                                                 blkdec_t[:, hl:hl + 1].to_broadcast([P, HD]))
                            nc.vector.tensor_add(kv_all[:, b + 1, :], kv_all[:, b + 1, :], c_ps[:])

                        qk_ps = []
                        for no in range(2):
                            qk = ap_.tile([P, BLOCK], F32, tag="qk")
                            nc.tensor.matmul(qk[:], qkv_c[:, 1, no * P:(no + 1) * P],
                                             qkv_c[:, 0, :], start=True, stop=True)
                            qk_ps.append(qk)
                        qkTm = asml.tile([P, 2, BLOCK], BF16, tag="qkm")
                        for no in range(2):
                            nc.vector.tensor_mul(qkTm[:, no, :], qk_ps[no][:], maskT_t[:, hl, no, :])
                        qdT = asml.tile([P, BLOCK], F32R, tag="qdT")
                        nc.vector.tensor_mul(qdT[:], qkv_c[:, 0, :], qdec_t[:, hl, :])

                        o_ps = ap_.tile([P, BLOCK], F32, tag="o")
                        nc.tensor.matmul(o_ps[:], kdvn[:, 1, 0, :], qkTm[:, 0, :], start=True, stop=False)
                        nc.tensor.matmul(o_ps[:], kdvn[:, 1, 1, :], qkTm[:, 1, :], start=False, stop=False)
                        nc.tensor.matmul(o_ps[:], kv_all[:, b, :], qdT[:], start=False, stop=True)
                        nc.any.tensor_copy(hiddenT_h[:, msl], o_ps[:])

                    # ssq partial + gated Y
                    sq_h = ah.tile([P, SEQ], BF16, tag="sq")
                    nc.vector.tensor_mul(sq_h[:], hiddenT_h[:], hiddenT_h[:])
                    for sc in range(SB):
                        sp = ap_.tile([P, P], F32, tag="C")
                        nc.tensor.matmul(sp[:], sq_h[:, sc * P:(sc + 1) * P], ones_t[:],
                                         start=True, stop=True)
                        if hl == 0:
                            nc.vector.tensor_copy(ssq_acc[:, sc:sc + 1], sp[:, 0:1])
                        else:
                            nc.vector.tensor_add(ssq_acc[:, sc:sc + 1], ssq_acc[:, sc:sc + 1], sp[:, 0:1])
                    if DBG:
                        nc.sync.dma_start(
                            dbg_hidT.ap().rearrange("(h p) s -> p h s", p=P)[:, hl, :], hiddenT_h[:])
                    y_h = ah.tile([P, SEQ], BF16, tag="y")
                    nc.vector.tensor_mul(y_h[:], hiddenT_h[:], gate_h[:])
                    for st in range(W):
                        nc.sync.dma_start(a2a_in[st, hl * P:(hl + 1) * P, :],
                                          y_h[:, st * SSH:(st + 1) * SSH])
                        if DBG:
                            nc.sync.dma_start(dbg_y.ap()[st, hl * P:(hl + 1) * P, :],
                                              y_h[:, st * SSH:(st + 1) * SSH])

            nc.sync.dma_start(ssq_in.rearrange("(a p) -> p a", p=P), ssq_acc[:])
            if DBG:
                nc.sync.dma_start(dbg_ssq.ap().rearrange("(a p) -> p a", p=P), ssq_acc[:])

            # ================= collectives ======================================
            if PHASES == "att":
                return _finish(nc)
            NOCOLL = bool(int(os.environ.get("KERNEL_NOCOLL", "0")))
            if NOCOLL:
                a2a_out = a2a_in
                ssq_out = ssq_in[:SSH]
            else:
                nc.gpsimd.collective_compute(
                    "AllToAll", ALU.bypass, replica_groups=[list(range(W))],
                    ins=[a2a_in.opt()], outs=[a2a_out.opt()])
                nc.gpsimd.collective_compute(
                    "ReduceScatter", ALU.add, replica_groups=[list(range(W))],
                    ins=[ssq_in.opt()], outs=[ssq_out.opt()])

            # rsqrt(var + eps)
            sq_raw = const.tile([P, SSH // P], F32)
            nc.sync.dma_start(sq_raw[:], ssq_out.rearrange("(i p) -> p i", p=P))
            t1 = const.tile([P, SSH // P], F32)
            nc.scalar.activation(t1[:], sq_raw[:], AF.Sqrt, bias=eps_t[:], scale=1.0 / INNER)
            nc.vector.reciprocal(rsq[:], t1[:])
            if DBG:
                nc.sync.dma_start(dbg_rsq.ap()[:], rsq[:])

            # ================= out projection (fp32r, seq-sharded) ==============
            with tc.tile_pool(name="oa", bufs=1) as oa, \
                 tc.tile_pool(name="ow", bufs=5) as ow, \
                 tc.tile_pool(name="oo", bufs=4) as oo, \
                 tc.tile_pool(name="op", bufs=4, space="PSUM") as opp:
                a2a_t = oa.tile([P, KO, SSH], BF16)
                a2a_v = a2a_out.rearrange("r (jo p) s -> p (r jo) s", p=P)
                for q in range(4):
                    nc.sync.dma_start(a2a_t[:, q * (KO // 4):(q + 1) * (KO // 4), :],
                                      a2a_v[:, q * (KO // 4):(q + 1) * (KO // 4), :])
                if DBG:
                    nc.sync.dma_start(dbg_a2a.ap().rearrange("r (jo p) s -> p (r jo) s", p=P), a2a_t[:])
                woutT_v = woutT.ap().rearrange("(ko p) o -> p ko o", p=P)
                out_v = out.ap().rearrange("(mo p) o -> p mo o", p=P)
                KC = KO // 4   # 8 k-subtiles per weight chunk
                for nt in range(H // 512):
                    osl = slice(nt * 512, (nt + 1) * 512)
                    w_ts = []
                    for q in range(4):
                        w_t = ow.tile([P, KC, 512], BF16, tag="w")
                        nc.sync.dma_start(w_t[:], woutT_v[:, q * KC:(q + 1) * KC, osl])
                        w_ts.append(w_t)
                    for mt in range(SSH // P):
                        ps = opp.tile([P, 512], F32, tag="po")
                        for kk in range(KO):
                            nc.tensor.matmul(ps[:], a2a_t[:, kk, mt * P:(mt + 1) * P],
                                             w_ts[kk // KC][:, kk % KC, :],
                                             start=(kk == 0), stop=(kk == KO - 1))
                        o_sb = oo.tile([P, 512], F32, tag="ot")
                        nc.vector.tensor_mul(o_sb[:], ps[:],
                                             rsq[:, mt:mt + 1].to_broadcast([P, 512]))
                        nc.sync.dma_start(out_v[:, mt, osl], o_sb[:])

    nc.compile()
    return nc


def _host_prep(inputs):
    x = np.asarray(inputs["x"], np.float32)
    w_qkv = np.asarray(inputs["w_qkv"], np.float32)
    w_gate = np.asarray(inputs["w_gate"], np.float32)
    w_out = np.asarray(inputs["w_out"], np.float32)
    norm_weight = np.asarray(inputs["norm_weight"], np.float32)
    kv_cache = np.asarray(inputs["kv_cache"], np.float32)
    slope = np.asarray(inputs["slope"], np.float32)

    bf = ml_dtypes.bfloat16
    xT_bf = np.ascontiguousarray(x.T).astype(bf)
    woutT = np.ascontiguousarray((w_out * norm_weight[None, :]).T).astype(bf)
    ident = np.eye(P, dtype=np.float32)
    ones = np.ones((P, P), ml_dtypes.bfloat16)

    in_maps = []
    for c in range(W):
        sl = slope[c * HPC:(c + 1) * HPC]                     # [4]
        m0 = np.arange(BLOCK, dtype=np.float32)              # 0-based position in block
        # qdec[p, hl, m] = exp(-s*(m+1)) replicated over partitions
        qd = np.exp(-sl[:, None] * (m0[None, :] + 1.0))      # [4, 256]
        qdec_a = np.broadcast_to(qd[None], (P, HPC, BLOCK)).astype(np.float32).copy()
        # kdec[p, hl, no] = exp(-s*(BLOCK - (no*128+p+1)))
        n0 = (np.arange(2)[None, :] * P + np.arange(P)[:, None]).astype(np.float32)  # [128,2]
        kd = np.exp(-sl[None, None, :] * (BLOCK - (n0[:, :, None] + 1.0)))           # [128,2,4]
        kdec_a = np.ascontiguousarray(kd.transpose(0, 2, 1)).astype(np.float32)      # [128,4,2]
        # maskT[p, hl, no, m] = exp(-s*(m - n)) if m>=n else 0   (0-based n = no*128+p)
        nfull = n0[:, :, None]                                # [128,2,1]
        diff = m0[None, None, :] - nfull                      # [128,2,256]
        dif4 = diff[..., None]                                # [128,2,256,1]
        mask = np.where(dif4 >= 0,
                        np.exp(-sl[None, None, None, :] * np.maximum(dif4, 0.0)),
                        0.0)                                  # [128,2,256,4]
        maskT_a = np.ascontiguousarray(mask.transpose(0, 3, 1, 2)).astype(np.float32)        # [128,4,2,256]
        blkdec_a = np.broadcast_to(np.exp(-sl * BLOCK)[None], (P, HPC)).astype(np.float32).copy()

        in_maps.append({
            "xT": xT_bf,
            "wqkvT": np.ascontiguousarray(w_qkv[MPC * c:MPC * (c + 1)].T).astype(bf),
            "wgateT": np.ascontiguousarray(w_gate[JPC * c:JPC * (c + 1)].T).astype(bf),
            "woutT": woutT,
            "qdec": qdec_a,
            "kdec": kdec_a,
            "maskT": maskT_a,
            "blkdec": blkdec_a,
            "ident_r": ident,
            "ones_r": ones,
            "eps_b": np.full((P, 1), EPS, np.float32),
            "kv0": np.ascontiguousarray(kv_cache[HPC * c:HPC * (c + 1)]),
        })
    return in_maps


_CACHE = {}


def _get_program():
    if "nc" not in _CACHE:
        _CACHE["nc"] = _build_program()
    return _CACHE["nc"]


def kernel(**inputs):
    nc = _get_program()
    in_maps = _host_prep(inputs)
    trace = bool(int(os.environ.get("KERNEL_TRACE", "0")))
    res = run_bass_kernel_spmd(nc, in_maps, core_ids=list(range(W)), trace=trace)
    _CACHE["last_results"] = res
    out = np.concatenate([res.results[c]["out"] for c in range(W)], axis=0)
    return out.astype(np.float32)
